# revision 1
# baseline (speedup 1.0000x reference)
"""Trainium2 Bass kernel for nn_LocalState_1580547972191 (sparse_attention).

Contract: kernel(**inputs) takes FULL unsharded inputs (as from setup_inputs()),
returns FULL output [4, 512, 2048] f32. Internally shards across 8 NeuronCores:
core = (b, hg) with b = batch, hg = head-group (heads {2hg, 2hg+1}).

Algorithm (per core), validated against the reference in fp64/fp32:
- The decay bias -g(s)|t-s| with g(s) >= ~0.28 makes attention effectively
  banded: weights for |t-s| > 256 are < e^-70 relative -> exactly 0 in fp32.
  Each 128-query block attends to a 640-wide key window (128-aligned).
- Freq bias cos(2*pi*(t-s)/p) = cos_p(t)cos_p(s) + sin_p(t)sin_p(s) is rank-2:
  folded into the QK^T matmul via 8 augmented rows.
- Decay bias applied as one fused DVE op: S2 = (D * (-g_p)) + S where D is a
  host-precomputed |t-s| pattern (5 distinct patterns) with the diagonal
  entries set to 1e4 (folds the eye-mask: exp(-1e4*g) = 0 = exp(-100)/sigma).
- No-max softmax: logits bounded (~15), so exp without max subtraction is
  safe in f32; sigma accumulated by the Exp activation's accum_out.
- PV needs W[t,s]; W'[s,t] tiles are transposed via the DMA xbar engines.
- time_sig recovered from 8 augmented content rows (cos/sin) post-PV; the
  cos(s)*cc + sin(s)*ss pair-sum is folded into the proj matmul by
  duplicating the tsig columns of W_proj.
- proj partial computed on-core; host sums the two head-group partials.
  Residual x, b_proj, and W_proj@b_content folded in on the hg=0 core.
"""
import math
import sys

sys.path.insert(0, "/opt/trn_rl_repo")

import ml_dtypes
import numpy as np

HEADS, NF, ND = 4, 4, 4
B, C, T = 4, 512, 2048
NBLK, WIN = 16, 384
DIAG_BIG = 1.0e4
BF16 = ml_dtypes.bfloat16

_CACHE = {}


def _w0_of_block(i):
    return 128 * min(max(i - 1, 0), 13)


def _tt_first_block(tt):
    for i in range(NBLK):
        base = min(max(i - 2, 0), 11)
        if base <= tt <= base + 4:
            return i
    raise AssertionError


def _TL(pool, shape, dtype, tag):
    return pool.tile(shape, dtype, name=tag, tag=tag)


def _build_nc():
    import concourse.mybir as mybir
    import concourse.tile as tile
    from concourse import bacc

    dt = mybir.dt
    f32, bf16 = dt.float32, dt.bfloat16
    Alu = mybir.AluOpType
    Act = mybir.ActivationFunctionType

    nc = bacc.Bacc("TRN2", target_bir_lowering=False, debug=False, num_devices=8)

    # ---- DRAM I/O (per-core shards, host-prepared) ----
    xb_d = nc.dram_tensor("xb", [C, T], bf16, kind="ExternalInput")
    wqkc_d = nc.dram_tensor("wqkc", [C, 792], bf16, kind="ExternalInput")
    smalls_d = nc.dram_tensor("smalls", [128, 16], f32, kind="ExternalInput")
    cs_d = nc.dram_tensor("cs", [8, T], bf16, kind="ExternalInput")
    csT_d = nc.dram_tensor("csT", [T, 8], bf16, kind="ExternalInput")
    d5_d = nc.dram_tensor("d5", [128, 3, WIN], f32, kind="ExternalInput")
    wp12_d = nc.dram_tensor("wp12", [256, C], bf16, kind="ExternalInput")
    wp3_d = nc.dram_tensor("wp3d", [16, C], bf16, kind="ExternalInput")
    out_d = nc.dram_tensor("out", [C, T], f32, kind="ExternalOutput")

    with tile.TileContext(nc) as tc:
        sing = tc.alloc_tile_pool(name="sing", bufs=1)
        work = tc.alloc_tile_pool(name="work", bufs=4)
        outp = tc.alloc_tile_pool(name="outp", bufs=2)
        ps_s = tc.alloc_tile_pool(name="ps_s", bufs=3, space="PSUM")
        ps_pv = tc.alloc_tile_pool(name="ps_pv", bufs=3, space="PSUM")
        ps_sm = tc.alloc_tile_pool(name="ps_sm", bufs=2, space="PSUM")

        # ---- load persistent inputs ----
        xb = [_TL(sing, [128, T], bf16, tag=f"xb{k}") for k in range(4)]
        wqkc = [_TL(sing, [128, 792], bf16, tag=f"wqkc{k}") for k in range(4)]
        for k in range(4):
            nc.sync.dma_start(out=wqkc[k], in_=wqkc_d[128 * k:128 * (k + 1), :])
            nc.sync.dma_start(out=xb[k], in_=xb_d[128 * k:128 * (k + 1), :])
        wqT = [w[:, 0:256] for w in wqkc]
        wkT = [w[:, 256:512] for w in wqkc]
        wcT = [w[:, 512:768] for w in wqkc]
        wfdT = [w[:, 768:792] for w in wqkc]
        smalls = _TL(sing, [128, 16], f32, tag="smalls")
        nc.sync.dma_start(out=smalls, in_=smalls_d[:, :])
        bq_sb = smalls[:, 0:2]
        bk_sb = smalls[:, 2:4]
        beff_sb = smalls[:, 4:8]
        bf_sb = [smalls[0:8, 8:9], smalls[0:8, 9:10]]
        bqd_sb = smalls[0:8, 10:11]
        gco_sb = smalls[0:8, 11:13]
        cs_sb = _TL(sing, [8, T], bf16, tag="cs")
        nc.sync.dma_start(out=cs_sb, in_=cs_d[:, :])
        d5_sb = _TL(sing, [128, 3, WIN], f32, tag="d5")
        nc.sync.dma_start(out=d5_sb, in_=d5_d[:, :, :])
        wp12_sb = [_TL(sing, [128, C], bf16, tag=f"wp{k}") for k in range(2)]
        for k in range(2):
            nc.sync.dma_start(out=wp12_sb[k], in_=wp12_d[128 * k:128 * (k + 1), :])
        wp3_sb = [_TL(sing, [8, C], bf16, tag=f"wp3_{h}") for h in range(2)]
        for h in range(2):
            nc.sync.dma_start(out=wp3_sb[h], in_=wp3_d[8 * h:8 * (h + 1), :])

        # contentT aux columns (cos/sin by t) straight from DRAM
        CT_sb = _TL(sing, [128, NBLK, 264], bf16, tag="CT")
        nc.sync.dma_start(
            out=CT_sb[:, :, 256:264],
            in_=csT_d.ap().rearrange("(tt p) c -> p tt c", p=128),
        )

        # ---- interleaved projection chunks + attention block groups ----
        Q_sb = [_TL(sing, [128, T], bf16, tag=f"Q{h}") for h in range(2)]
        K_sb = [_TL(sing, [128, T], bf16, tag=f"K{h}") for h in range(2)]
        fqh = [_TL(sing, [8, T], bf16, tag=f"fqh{h}") for h in range(2)]
        qd_sb = _TL(sing, [8, T], f32, tag="qd")
        sig_sb = _TL(sing, [8, T], f32, tag="sig")
        gneg_sb = _TL(sing, [128, 2 * NBLK], f32, tag="gneg")
        Qaux = [_TL(sing, [8, T], bf16, tag=f"Qaux{h}") for h in range(2)]
        Res = [_TL(sing, [128, T], bf16, tag=f"Res{h}") for h in range(2)]
        Tsaux = [_TL(sing, [8, T], bf16, tag=f"Tsaux{h}") for h in range(2)]
        prod8 = [_TL(sing, [8, T], bf16, tag=f"prod8{h}") for h in range(2)]
        WnT = _TL(sing, [128, NBLK, 2, WIN], bf16, tag="WnT")
        pat_of = lambda i: {0: 0, 15: 2}.get(i, 1)

        def emit_proj_chunk(n):
            cols = slice(512 * n, 512 * (n + 1))
            for h in range(2):
                pq = _TL(ps_pv, [128, 512], f32, tag="proj")
                for k in range(4):
                    nc.tensor.matmul(pq, wqT[k][:, 128 * h:128 * (h + 1)],
                                     xb[k][:, cols], start=(k == 0), stop=(k == 3))
                nc.scalar.activation(out=Q_sb[h][:, cols], in_=pq,
                                     func=Act.Identity, bias=bq_sb[:, h:h + 1])
                pk = _TL(ps_pv, [128, 512], f32, tag="proj")
                for k in range(4):
                    nc.tensor.matmul(pk, wkT[k][:, 128 * h:128 * (h + 1)],
                                     xb[k][:, cols], start=(k == 0), stop=(k == 3))
                nc.scalar.activation(out=K_sb[h][:, cols], in_=pk,
                                     func=Act.Identity, bias=bk_sb[:, h:h + 1])
                pf = _TL(ps_sm, [8, 512], f32, tag="sm")
                for k in range(4):
                    nc.tensor.matmul(pf, wfdT[k][:, 8 * h:8 * h + 8],
                                     xb[k][:, cols], start=(k == 0), stop=(k == 3))
                nc.scalar.activation(out=fqh[h][:, cols], in_=pf,
                                     func=Act.Identity, bias=bf_sb[h][:, 0:1])
            pd = _TL(ps_sm, [8, 512], f32, tag="sm")
            for k in range(4):
                nc.tensor.matmul(pd, wfdT[k][:, 16:24],
                                 xb[k][:, cols], start=(k == 0), stop=(k == 3))
            nc.scalar.activation(out=qd_sb[:, cols], in_=pd,
                                 func=Act.Identity, bias=bqd_sb[:, 0:1])
            for tt in range(4 * n, 4 * n + 4):   # contentT tiles
                rows = slice(128 * tt, 128 * (tt + 1))
                pc = _TL(ps_pv, [128, 256], f32, tag="proj")
                for k in range(4):
                    nc.tensor.matmul(pc, xb[k][:, rows], wcT[k],
                                     start=(k == 0), stop=(k == 3))
                nc.scalar.activation(out=CT_sb[:, tt, 0:256], in_=pc, func=Act.Copy)
            etmp = _TL(work, [8, 512], f32, tag="sigtmp")
            nc.scalar.activation(out=etmp, in_=qd_sb[:, cols],
                                 func=Act.Exp, scale=-1.0)
            ep1 = _TL(work, [8, 512], f32, tag="sigtmp2")
            nc.vector.tensor_scalar_add(ep1, etmp, 1.0)
            nc.vector.reciprocal(out=sig_sb[:, cols], in_=ep1)
            for i in range(4 * n, 4 * n + 4):
                pg = _TL(ps_sm, [128, 2], f32, tag="sm")
                nc.tensor.matmul(pg, sig_sb[:, 128 * i:128 * (i + 1)], gco_sb,
                                 start=True, stop=True)
                nc.vector.tensor_copy(out=gneg_sb[:, 2 * i:2 * i + 2], in_=pg)
            for h in range(2):
                nc.gpsimd.tensor_mul(Qaux[h][:, cols], cs_sb[:, cols],
                                     fqh[h][:, cols])

        def emit_block_pair(i):
            s0, w0 = 128 * i, _w0_of_block(i)
            base = min(max(i - 1, 0), 13)
            wn2 = _TL(work, [128, 2 * WIN], bf16, tag="Wn2")
            s2s, sigmas = [], []
            for h in range(2):
                sp = _TL(ps_s, [128, WIN], f32, tag="S")
                nc.tensor.matmul(sp, Q_sb[h][:, s0:s0 + 128],
                                 K_sb[h][:, w0:w0 + WIN], start=True, stop=False)
                nc.tensor.matmul(sp, Qaux[h][:, s0:s0 + 128],
                                 cs_sb[:, w0:w0 + WIN], start=False, stop=True)
                nc.vector.scalar_tensor_tensor(
                    out=sp, in0=d5_sb[:, pat_of(i), :],
                    scalar=gneg_sb[:, 2 * i + h:2 * i + h + 1],
                    in1=sp, op0=Alu.mult, op1=Alu.add)
                s2s.append(sp)
            wexps = []
            for h in range(2):
                wexp = _TL(work, [128, WIN], bf16, tag="Wexp")
                sigma = _TL(work, [128, 1], f32, tag="sigma")
                nc.scalar.activation(out=wexp, in_=s2s[h], func=Act.Exp,
                                     accum_out=sigma)
                wexps.append(wexp); sigmas.append(sigma)
            for h in range(2):
                recip = _TL(work, [128, 1], f32, tag="recip")
                nc.vector.reciprocal(out=recip, in_=sigmas[h])
                nc.vector.tensor_scalar_mul(wn2[:, WIN * h:WIN * (h + 1)],
                                            wexps[h], recip)
            nc.scalar.dma_start_transpose(
                out=WnT[:, i, :, :].rearrange("p hh (k e) -> p (hh k) e", e=128),
                in_=wn2)
            for h in range(2):
                om = _TL(ps_pv, [128, 128], f32, tag="proj")
                oa = _TL(ps_sm, [8, 128], f32, tag="sm")
                for j in range(3):
                    tt = base + j
                    rhs = WnT[:, i, h, 128 * j:128 * (j + 1)]
                    nc.tensor.matmul(om, CT_sb[:, tt, 128 * h:128 * (h + 1)],
                                     rhs, start=(j == 0), stop=(j == 2))
                    nc.tensor.matmul(oa, CT_sb[:, tt, 256:264],
                                     rhs, start=(j == 0), stop=(j == 2))
                nc.vector.tensor_copy(out=Res[h][:, s0:s0 + 128], in_=om)
                nc.scalar.activation(out=Tsaux[h][:, s0:s0 + 128], in_=oa,
                                     func=Act.Copy)

        def emit_out_ot(ot):
            osl = slice(128 * ot, 128 * (ot + 1))
            ob = _TL(outp, [128, T], f32, tag="ob")
            for n in range(4):
                cols = slice(512 * n, 512 * (n + 1))
                pp = _TL(ps_pv, [128, 512], f32, tag="proj")
                nc.tensor.matmul(pp, wp12_sb[0][:, osl], Res[0][:, cols],
                                 start=True, stop=False)
                nc.tensor.matmul(pp, wp12_sb[1][:, osl], Res[1][:, cols],
                                 start=False, stop=False)
                nc.tensor.matmul(pp, wp3_sb[0][:, osl], prod8[0][:, cols],
                                 start=False, stop=False)
                nc.tensor.matmul(pp, wp3_sb[1][:, osl], prod8[1][:, cols],
                                 start=False, stop=True)
                if n % 2 == 0:
                    nc.vector.tensor_scalar(
                        out=ob[:, cols], in0=pp, scalar1=beff_sb[:, ot:ot + 1],
                        scalar2=None, op0=Alu.add)
                else:
                    nc.scalar.activation(out=ob[:, cols], in_=pp,
                                         func=Act.Identity,
                                         bias=beff_sb[:, ot:ot + 1])
            nc.sync.dma_start(out=out_d[osl, :], in_=ob)

        for n in range(4):
            emit_proj_chunk(n)
        for i in range(NBLK):
            emit_block_pair(i)
        for n in range(4):
            cols = slice(512 * n, 512 * (n + 1))
            for h in range(2):
                nc.gpsimd.tensor_mul(prod8[h][:, cols], cs_sb[:, cols],
                                     Tsaux[h][:, cols])
        for ot in range(4):
            emit_out_ot(ot)

        for pool in (ps_sm, ps_pv, ps_s, outp, work, sing):
            pool.release()

    nc.compile()
    return nc


def _cos_sin():
    t = np.arange(T, dtype=np.float64)
    per = np.arange(1, NF + 1, dtype=np.float64)
    ang = 2 * math.pi * t[None, :] / per[:, None]
    return np.cos(ang).astype(np.float32), np.sin(ang).astype(np.float32)


def _d_patterns():
    d5 = np.empty((128, 3, WIN), np.float32)
    p = np.arange(128)[:, None]
    j = np.arange(WIN)[None, :]
    for k, off in enumerate((0, 128, 256)):
        d = np.abs(p + off - j).astype(np.float32)
        d[p + off == j] = DIAG_BIG
        d5[:, k, :] = d
    return d5


_COS, _SIN = _cos_sin()
_D5 = _d_patterns()


def _prep_core_inputs(inputs, b, hg):
    f32 = np.float32
    x_b = np.ascontiguousarray(np.asarray(inputs["x"])[b], dtype=f32)
    hsl = slice(hg * 256, (hg + 1) * 256)
    fsl = slice(hg * 8, (hg + 1) * 8)
    s = f32(1.0 / math.sqrt(128.0))
    cosT, sinT = _COS, _SIN

    Wq = np.asarray(inputs["W_query"], f32)[hsl] * s
    bq = np.asarray(inputs["b_query"], f32)[hsl] * s
    Wk = np.asarray(inputs["W_key"], f32)[hsl]
    bk = np.asarray(inputs["b_key"], f32)[hsl]
    Wc = np.asarray(inputs["W_content"], f32)[hsl]
    Wf = np.asarray(inputs["W_qfreq"], f32)[fsl] * f32(0.5)
    bf = np.asarray(inputs["b_qfreq"], f32)[fsl] * f32(0.5)
    Wd = np.asarray(inputs["W_qdecay"], f32)[fsl]
    bd = np.asarray(inputs["b_qdecay"], f32)[fsl]
    Wp = np.asarray(inputs["W_proj"], f32)
    Wp_hg = Wp[:, hg * 264:(hg + 1) * 264]

    # wfdT cols: [fq_h0 x2 (dup), fq_h1 x2 (dup), qd_h0, qd_h1]
    wfd = np.concatenate([Wf[0:4], Wf[0:4], Wf[4:8], Wf[4:8],
                          Wd[0:4], Wd[4:8]], axis=0)  # [24, 512]

    gco = np.zeros((8, 2), f32)
    dvec = -(np.arange(1, ND + 1, dtype=f32) / 4)
    gco[0:4, 0] = dvec
    gco[4:8, 1] = dvec

    wp12 = np.concatenate([Wp_hg[:, 0:128].T, Wp_hg[:, 132:260].T], axis=0)
    # tsig proj cols duplicated (cos part rows 0-3, sin part rows 4-7)
    wp3d = np.concatenate([Wp_hg[:, 128:132].T, Wp_hg[:, 128:132].T,
                           Wp_hg[:, 260:264].T, Wp_hg[:, 260:264].T], axis=0)

    if hg == 0:
        b_eff = np.asarray(inputs["b_proj"], f32).copy()
        bc = np.asarray(inputs["b_content"], f32)
        for h in range(HEADS):
            b_eff += Wp[:, 132 * h:132 * h + 128] @ bc[128 * h:128 * h + 128]
    else:
        b_eff = np.zeros(C, f32)
    beff = np.ascontiguousarray(b_eff.reshape(4, 128).T)  # b_eff[128*ot+p] -> [p, ot]

    wqkc = np.concatenate([Wq.T, Wk.T, Wc.T, wfd.T], axis=1)  # [512, 792]
    smalls = np.zeros((128, 16), f32)
    smalls[:, 0:2] = bq.reshape(2, 128).T
    smalls[:, 2:4] = bk.reshape(2, 128).T
    smalls[:, 4:8] = beff
    smalls[0:8, 8] = np.concatenate([bf[0:4], bf[0:4]])
    smalls[0:8, 9] = np.concatenate([bf[4:8], bf[4:8]])
    smalls[0:8, 10] = np.concatenate([bd[0:4], bd[4:8]])
    smalls[0:8, 11:13] = gco
    return {
        "xb": x_b.astype(BF16),
        "wqkc": np.ascontiguousarray(wqkc).astype(BF16),
        "smalls": smalls,
        "cs": np.concatenate([cosT, sinT], 0).astype(BF16),
        "csT": np.ascontiguousarray(np.concatenate([cosT, sinT], 0).T).astype(BF16),
        "d5": _D5,
        "wp12": np.ascontiguousarray(wp12).astype(BF16),
        "wp3d": np.ascontiguousarray(wp3d).astype(BF16),
    }


def get_nc():
    if "nc" not in _CACHE:
        _CACHE["nc"] = _build_nc()
    return _CACHE["nc"]


def make_in_maps(inputs):
    return [_prep_core_inputs(inputs, c // 2, c % 2) for c in range(8)]


def kernel(**inputs):
    from concourse.bass_utils import run_bass_kernel_spmd

    nc = get_nc()
    in_maps = make_in_maps(inputs)
    res = run_bass_kernel_spmd(nc, in_maps, core_ids=list(range(8)))
    x = np.asarray(inputs["x"], np.float32)
    out = np.empty((B, C, T), np.float32)
    for b in range(B):
        np.add(res.results[2 * b]["out"], res.results[2 * b + 1]["out"], out=out[b])
        out[b] += x[b]
    return out



# revision 18
# speedup vs baseline: 1.0925x; 1.0925x over previous
"""Trainium2 Bass kernel for nn_LocalState_1580547972191 (sparse_attention).

Contract: kernel(**inputs) takes FULL unsharded inputs (as from setup_inputs()),
returns FULL output [4, 512, 2048] f32. Internally shards across 8 NeuronCores:
core = (b, hg) with b = batch, hg = head-group (heads {2hg, 2hg+1}).

v2 design (cost-model-guided rewrite of the v1 baseline):
- Banded attention, 128-query blocks, 256-wide key window at 64-offset
  (w0 = clamp(128i-64, 0, 1792)); out-of-window softmax mass < 2e-8.
- Freq bias rank-2 folded into QK matmul via 8 augmented rows (cos/sin).
- Decay bias: one fused DVE/Pool scalar_tensor_tensor per (block, head):
  S += D * gneg, D = |t-s| pattern (3 patterns) with 1e4 on the diagonal
  (folds the eye-mask). gneg from a tanh-based sigmoid (tanh is in every
  activation table; avoids an act-table reload).
- fq/qd projections fused into ONE 24-row matmul per chunk (cost model:
  matmul cost ~ output free size only).
- exp without max-subtraction (logits bounded); one exp per block covers
  both heads ([128,2,256] PSUM spanning 2 banks).
- W^T via batched DMA transposes (4 blocks per instruction); zero-padded
  384-wide slabs keep PV contraction 128-aligned.
- PV computed transposed: omT[s,c] = sum_t W[t,s] content[c,t] with an
  fused aux rhs [csT(8) | ones | content(128)] so each series also yields
  time-sig partials AND sigma (the softmax denominator) for free (N=137).
- Normalization (x 1/sigma) applied per-partition on omT/oaT, then omT is
  DMA-transposed back into Res for the output projection.
- Output projection: content (2x K=128) + [tsig|bias] (K=17, ones row
  folds b_proj and W_proj@b_content) = 3 matmuls per (ot, n).
- bf16 partial outputs; host sums the two head-group partials + x.
- PE warmup matmuls bridge the p-state ramp across the initial DMA loads.
"""
import math
import sys

sys.path.insert(0, "/opt/trn_rl_repo")

import ml_dtypes
import numpy as np

HEADS, NF, ND = 4, 4, 4
B, C, T = 4, 512, 2048
NBLK, WIN = 16, 256
DIAG_BIG = 1.0e4
BF16 = ml_dtypes.bfloat16

_CACHE = {}

# bigb column layout (bf16, [128, XWCOLS])
XB0 = 0                  # xb [128, 4, 2048]
WQ0 = XB0 + 4 * 2048     # wqkc [128, 4, 792]
WP0 = WQ0 + 4 * 792      # wp12 [128, 2, 512]
CST0 = WP0 + 2 * 512     # csT dup [128, 16, 2, 8]
XWCOLS = CST0 + 16 * 16

# smallr column layout (f32r, [128, SRCOLS]): d5 + identity
D50 = 0
SR_I = D50 + 3 * 256
SRCOLS = SR_I + 128
# smallf column layout (f32, [128, SFCOLS])
SF_BQ = 0                # bq [128, 2] (scaled)
SF_BK = SF_BQ + 2        # bk [128, 2]
SF_FB = SF_BK + 2        # fbias16 [16, 1] at rows 0:16
SF_BD = SF_FB + 1        # 0.5*b_qdecay [8, 1] rows 0:8
SF_GCO = SF_BD + 1       # gco [8, 2] rows 0:8
SFCOLS = SF_GCO + 2

# CT layout: [auxA(9) | c_h0(128) | auxB(9) | c_h1(128)] = 274, pad 288
CT_A0, CT_C0, CT_A1, CT_C1, CTW = 0, 9, 137, 146, 288


def _w0_of_block(i):
    return min(max(128 * i - 64, 0), T - WIN)


def _off_of_block(i):
    # write offset of the active window inside the 384-wide padded slab
    return _w0_of_block(i) - 128 * (i - 1)


def _pat_of_block(i):
    return {0: 0, NBLK - 1: 2}.get(i, 1)


def _ks_of_block(i):
    return [k for k in range(3) if 0 <= i - 1 + k <= NBLK - 1]


def _TL(pool, shape, dtype, tag, **kw):
    return pool.tile(shape, dtype, name=tag, tag=tag, **kw)


def _build_nc():
    import concourse.mybir as mybir
    import concourse.tile as tile
    from concourse import bacc

    dt = mybir.dt
    f32, bf16 = dt.float32, dt.bfloat16
    Alu = mybir.AluOpType
    Act = mybir.ActivationFunctionType

    nc = bacc.Bacc("TRN2", target_bir_lowering=False, debug=False, num_devices=8)

    bigb_d = nc.dram_tensor("bigb", [128, XWCOLS], bf16, kind="ExternalInput")
    smallb_d = nc.dram_tensor("smallb", [17, 6656], bf16, kind="ExternalInput")
    smallf_d = nc.dram_tensor("smallf", [128, SFCOLS], f32, kind="ExternalInput")
    smallr_d = nc.dram_tensor("smallr", [128, SRCOLS], dt.float32r, kind="ExternalInput")
    out_d = nc.dram_tensor("out", [C, T], bf16, kind="ExternalOutput")

    with tile.TileContext(nc) as tc:
        sing = tc.alloc_tile_pool(name="sing", bufs=1)
        psS = tc.alloc_tile_pool(name="psS", bufs=2, space="PSUM")
        psB = tc.alloc_tile_pool(name="psB", bufs=2, space="PSUM")
        psP = tc.alloc_tile_pool(name="psP", bufs=2, space="PSUM")
        work = tc.alloc_tile_pool(name="work", bufs=4)

        # ---- persistent SBUF tiles ----
        XW = _TL(sing, [128, XWCOLS], bf16, tag="XW")
        CS = _TL(sing, [17, 6656], bf16, tag="CS")
        SF = _TL(sing, [128, SFCOLS], f32, tag="SF")
        SR = _TL(sing, [128, SRCOLS], dt.float32r, tag="SR")
        Q_sb = [_TL(sing, [128, T], bf16, tag=f"Q{h}") for h in range(2)]
        K_sb = [_TL(sing, [128, T], bf16, tag=f"K{h}") for h in range(2)]
        CT = _TL(sing, [128, NBLK, CTW], bf16, tag="CT")
        fqh16 = _TL(sing, [16, T], bf16, tag="fqh16")
        Qaux16 = _TL(sing, [16, T], bf16, tag="Qaux16")
        sig_sb = _TL(sing, [8, T], f32, tag="sig")
        gneg_sb = _TL(sing, [128, 2 * NBLK], f32, tag="gneg")
        recip_sb = _TL(sing, [128, 2 * NBLK], f32, tag="recip")
        wn2 = _TL(sing, [128, NBLK, 2, 384], bf16, tag="wn2")
        WnTu = _TL(sing, [128, NBLK, 2, 3, 128], bf16, tag="WnTu")
        OMPT = _TL(sing, [128, NBLK, 3, 128], bf16, tag="OMPT")
        RES3 = _TL(sing, [128, NBLK, 3, 128], bf16, tag="RES3")
        OB = _TL(sing, [128, 4, T], bf16, tag="OB")
        warm = _TL(sing, [128, 512], bf16, tag="warm")

        xb = [XW[:, XB0 + 2048 * k:XB0 + 2048 * (k + 1)] for k in range(4)]
        wqkc = [XW[:, WQ0 + 792 * k:WQ0 + 792 * (k + 1)] for k in range(4)]
        wqT = [w[:, 0:256] for w in wqkc]
        wkT = [w[:, 256:512] for w in wqkc]
        wcT = [w[:, 512:768] for w in wqkc]
        wfdT = [w[:, 768:792] for w in wqkc]
        wp12 = [XW[:, WP0 + 512 * h:WP0 + 512 * (h + 1)] for h in range(2)]
        cs16 = CS[0:16, 0:2048]
        csz = [CS[0:16, 2048:4096], CS[0:16, 4096:6144]]
        wp3cat = CS[0:17, 6144:6656]
        d5 = [SR[:, D50 + 256 * p:D50 + 256 * (p + 1)] for p in range(3)]
        bq = SF[:, SF_BQ:SF_BQ + 2]
        bk = SF[:, SF_BK:SF_BK + 2]
        fb16 = SF[0:16, SF_FB:SF_FB + 1]
        bdh = SF[0:8, SF_BD:SF_BD + 1]
        gco = SF[0:8, SF_GCO:SF_GCO + 2]
        I_sb = SR[:, SR_I:SR_I + 128]

        # ---- init: memsets (overlap the loads) + PE warmup ----
        nc.vector.memset(warm[:, :], 0.0)
        nc.gpsimd.memset(wn2[:, :, :, 0:64], 0.0)
        nc.gpsimd.memset(wn2[:, :, :, 320:384], 0.0)
        nc.vector.memset(wn2[:, 0, :, 64:128], 0.0)
        nc.vector.memset(wn2[:, NBLK - 1, :, 256:320], 0.0)
        nc.gpsimd.memset(CT[:, :, CT_A0 + 8:CT_A0 + 9], 1.0)
        nc.gpsimd.memset(CT[:, :, CT_A1 + 8:CT_A1 + 9], 1.0)
        nc.vector.memset(OMPT[:, :, 2, 16:17], 1.0)

        # warmup matmuls: keep PE continuously busy through the input DMAs so
        # the p-state is fully ramped when real work starts.
        for w in range(18):
            pw = _TL(psP, [128, 512], f32, tag="pP", padded_shape=[128, 512])
            nc.tensor.matmul(pw, warm[:, 0:128], warm, start=True, stop=True)

        # ---- loads (Q/K weights and first xb chunk first) ----
        wsrc = bigb_d.ap()[:, WQ0:WP0].rearrange("p (k c) -> p k c", k=4)
        wdst = XW[:, WQ0:WP0].rearrange("p (k c) -> p k c", k=4)
        nc.sync.dma_start(out=wdst[:, :, 0:512], in_=wsrc[:, :, 0:512])
        xbsrc = bigb_d.ap()[:, XB0:WQ0].rearrange("p (k t) -> p k t", k=4)
        xbdst = XW[:, XB0:WQ0].rearrange("p (k t) -> p k t", k=4)
        nc.sync.dma_start(out=xbdst[:, :, 0:512], in_=xbsrc[:, :, 0:512])
        nc.sync.dma_start(out=SF, in_=smallf_d[:, :])
        nc.sync.dma_start(out=SR, in_=smallr_d[:, :])
        nc.sync.dma_start(out=wdst[:, :, 512:792], in_=wsrc[:, :, 512:792])
        nc.sync.dma_start(out=XW[:, WP0:XWCOLS], in_=bigb_d[:, WP0:XWCOLS])
        nc.sync.dma_start(out=CS, in_=smallb_d[:, :])
        for n in range(1, 4):
            nc.sync.dma_start(out=xbdst[:, :, 512 * n:512 * (n + 1)],
                              in_=xbsrc[:, :, 512 * n:512 * (n + 1)])

        csT_src = XW[:, CST0:CST0 + 256].rearrange("p (tt c) -> p tt c", tt=16)
        nc.vector.tensor_copy(out=CT[:, :, CT_A0:CT_A0 + 8], in_=csT_src[:, :, 0:8])
        nc.gpsimd.tensor_copy(out=CT[:, :, CT_A1:CT_A1 + 8], in_=csT_src[:, :, 8:16])

        # ---- phase A pieces ----
        def emit_qk(n):
            cols = slice(512 * n, 512 * (n + 1))
            for h in range(2):
                pq = _TL(psP, [128, 512], f32, tag="pP", padded_shape=[128, 512])
                for k in range(4):
                    nc.tensor.matmul(pq, wqT[k][:, 128 * h:128 * (h + 1)],
                                     xb[k][:, cols], start=(k == 0), stop=(k == 3))
                if h == 0:
                    nc.scalar.activation(out=Q_sb[h][:, cols], in_=pq,
                                         func=Act.Identity, bias=bq[:, h:h + 1])
                else:
                    nc.vector.tensor_scalar(out=Q_sb[h][:, cols], in0=pq,
                                            scalar1=bq[:, h:h + 1], scalar2=None,
                                            op0=Alu.add)
                pk = _TL(psP, [128, 512], f32, tag="pP", padded_shape=[128, 512])
                for k in range(4):
                    nc.tensor.matmul(pk, wkT[k][:, 128 * h:128 * (h + 1)],
                                     xb[k][:, cols], start=(k == 0), stop=(k == 3))
                if h == 1:
                    nc.scalar.activation(out=K_sb[h][:, cols], in_=pk,
                                         func=Act.Identity, bias=bk[:, h:h + 1])
                else:
                    nc.vector.tensor_scalar(out=K_sb[h][:, cols], in0=pk,
                                            scalar1=bk[:, h:h + 1], scalar2=None,
                                            op0=Alu.add)

        def emit_fd(n):
            cols = slice(512 * n, 512 * (n + 1))
            pfq = _TL(psB, [16, 512], f32, tag="pB", padded_shape=[128, 512])
            for k in range(4):
                nc.tensor.matmul(pfq, wfdT[k][:, 0:16], xb[k][:, cols],
                                 start=(k == 0), stop=(k == 3))
            nc.scalar.activation(out=fqh16[:, cols], in_=pfq,
                                 func=Act.Identity, bias=fb16)
            pqd = _TL(psB, [8, 512], f32, tag="pB", padded_shape=[128, 512])
            for k in range(4):
                nc.tensor.matmul(pqd, wfdT[k][:, 16:24], xb[k][:, cols],
                                 start=(k == 0), stop=(k == 3))
            nc.scalar.activation(out=sig_sb[:, cols], in_=pqd,
                                 func=Act.Tanh, bias=bdh, scale=0.5)
            nc.gpsimd.tensor_tensor(out=Qaux16[:, cols], in0=cs16[:, cols],
                                    in1=fqh16[:, cols], op=Alu.mult)

        def emit_ct(tt, j):
            rows = slice(128 * tt, 128 * (tt + 1))
            pc = _TL(psP, [128, 256], f32, tag="pP", padded_shape=[128, 512])
            for k in range(4):
                nc.tensor.matmul(pc, xb[k][:, rows], wcT[k],
                                 start=(k == 0), stop=(k == 3))
            if j % 2 == 0:
                nc.vector.tensor_copy(out=CT[:, tt, CT_C0:CT_C0 + 128], in_=pc[:, 0:128])
                nc.vector.tensor_copy(out=CT[:, tt, CT_C1:CT_C1 + 128], in_=pc[:, 128:256])
            else:
                nc.scalar.activation(out=CT[:, tt, CT_C0:CT_C0 + 128],
                                     in_=pc[:, 0:128], func=Act.Copy)
                nc.scalar.activation(out=CT[:, tt, CT_C1:CT_C1 + 128],
                                     in_=pc[:, 128:256], func=Act.Copy)

        def emit_gneg(i):
            pg = _TL(psB, [128, 2], f32, tag="pB", padded_shape=[128, 512])
            nc.tensor.matmul(pg, sig_sb[:, 128 * i:128 * (i + 1)], gco,
                             start=True, stop=True)
            nc.vector.tensor_scalar(out=gneg_sb[:, 2 * i:2 * i + 2], in0=pg,
                                    scalar1=-1.25, scalar2=None, op0=Alu.add)

        # ---- phase B: attention S + exp for one block ----
        def emit_diag(i):
            dg = _TL(work, [128, 2, 128], dt.float32r, tag="diag")
            for h in range(2):
                nc.gpsimd.tensor_scalar(out=dg[:, h, :], in0=I_sb,
                                        scalar1=gneg_sb[:, 2 * i + h:2 * i + h + 1],
                                        scalar2=None, op0=Alu.mult)
            return dg

        def emit_S(i, dg):
            s0, w0 = 128 * i, _w0_of_block(i)
            off = _off_of_block(i)
            s2t = _TL(psS, [128, 2, 512], f32, tag="pS", padded_shape=[128, 2, 512])
            for h in range(2):
                nc.tensor.matmul(s2t[:, h, 0:WIN], Q_sb[h][:, s0:s0 + 128],
                                 K_sb[h][:, w0:w0 + WIN], start=True, stop=False)
                nc.tensor.matmul(s2t[:, h, 0:WIN], Qaux16[:, s0:s0 + 128],
                                 csz[h][:, w0:w0 + WIN], start=False, stop=False)
                nc.tensor.matmul(s2t[:, h, 0:WIN], dg[:, h, :],
                                 d5[_pat_of_block(i)], start=False, stop=True)
            nc.scalar.activation(out=wn2[:, i, :, off:off + WIN],
                                 in_=s2t[:, :, 0:WIN], func=Act.Exp)

        # ---- phase B: PV (transposed) for one block ----
        def emit_pv(i):
            ks = _ks_of_block(i)
            pomh = []
            for h in range(2):
                pom = _TL(psB, [128, 137], f32, tag="pB", padded_shape=[128, 512])
                pomh.append(pom)
                rhs0 = CT_A0 if h == 0 else CT_A1
                for j, k in enumerate(ks):
                    nc.tensor.matmul(
                        pom, WnTu[:, i, h, k, :],
                        CT[:, i - 1 + k, rhs0:rhs0 + 137],
                        start=(j == 0), stop=(j == len(ks) - 1))
            for h in range(2):
                nc.vector.reciprocal(out=recip_sb[:, 2 * i + h:2 * i + h + 1],
                                     in_=pomh[h][:, 8:9])
            nc.vector.tensor_scalar(out=OMPT[:, i, 0, :], in0=pomh[0][:, 9:137],
                                     scalar1=recip_sb[:, 2 * i:2 * i + 1],
                                     scalar2=None, op0=Alu.mult)
            nc.scalar.activation(out=OMPT[:, i, 1, :], in_=pomh[1][:, 9:137],
                                 func=Act.Identity,
                                 scale=recip_sb[:, 2 * i + 1:2 * i + 2])
            for h in range(2):
                nc.vector.scalar_tensor_tensor(
                    out=OMPT[:, i, 2, 8 * h:8 * h + 8], in0=pomh[h][:, 0:8],
                    scalar=recip_sb[:, 2 * i + h:2 * i + h + 1],
                    in1=CT[:, i, CT_A0:CT_A0 + 8], op0=Alu.mult, op1=Alu.mult)

        def emit_wn_transpose2(p):
            nc.sync.dma_start_transpose(
                out=WnTu[:, 2 * p:2 * p + 2, :, :, :].rearrange(
                    "p b hh k e -> p (b hh k) e"),
                in_=wn2[:, 2 * p:2 * p + 2, :, :])

        def emit_post_transposes(p):
            nc.sync.dma_start_transpose(
                out=RES3[:, 2 * p:2 * p + 2, :, :],
                in_=OMPT[:, 2 * p:2 * p + 2, :, :])

        # ---- phase C: output projection for block pair p ----
        def emit_out(p):
            cols = slice(256 * p, 256 * (p + 1))
            for ot in range(4):
                osl = slice(128 * ot, 128 * (ot + 1))
                pp = _TL(psP, [128, 256], f32, tag="pP", padded_shape=[128, 512])
                nc.tensor.matmul(pp, wp12[0][:, osl], RES3[:, 2 * p:2 * p + 2, 0, :],
                                 start=True, stop=False)
                nc.tensor.matmul(pp, wp12[1][:, osl], RES3[:, 2 * p:2 * p + 2, 1, :],
                                 start=False, stop=False)
                nc.tensor.matmul(pp, wp3cat[:, osl],
                                 RES3[0:17, 2 * p:2 * p + 2, 2, :],
                                 start=False, stop=True)
                if ot % 2 == 0:
                    nc.vector.tensor_copy(out=OB[:, ot, cols], in_=pp)
                else:
                    nc.scalar.activation(out=OB[:, ot, cols], in_=pp, func=Act.Copy)

        def emit_store(g):
            cs_ = slice(512 * g, 512 * (g + 1))
            nc.scalar.dma_start(
                out=out_d.ap().rearrange("(ot p) t -> p ot t", p=128)[:, :, cs_],
                in_=OB[:, :, cs_])

        NP = NBLK // 2
        # prologue
        emit_qk(0)
        emit_fd(0)
        emit_ct(0, 0)
        emit_ct(1, 1)
        for i in range(4):
            emit_gneg(i)
        emit_qk(1)
        emit_fd(1)
        emit_ct(2, 2)
        emit_ct(3, 3)
        for i in range(4, 8):
            emit_gneg(i)
        dgq = [emit_diag(0), emit_diag(1)]
        for q in range(3):
            dgq.append(emit_diag(2 * q + 2))
            dgq.append(emit_diag(2 * q + 3))
            emit_S(2 * q, dgq[2 * q])
            emit_S(2 * q + 1, dgq[2 * q + 1])
            emit_wn_transpose2(q)
        for p in range(NP):
            if p == 0:
                emit_qk(2)
                emit_fd(2)
                for i in range(8, 12):
                    emit_gneg(i)
            if p == 2:
                emit_qk(3)
                emit_fd(3)
                for i in range(12, 16):
                    emit_gneg(i)
            if p < 6:
                emit_ct(2 * p + 4, 2 * p)
                emit_ct(2 * p + 5, 2 * p + 1)
            if p + 3 < NP:
                dgq.append(emit_diag(2 * p + 8)) if 2 * p + 8 < NBLK else None
                dgq.append(emit_diag(2 * p + 9)) if 2 * p + 9 < NBLK else None
                emit_S(2 * p + 6, dgq[2 * p + 6])
                emit_S(2 * p + 7, dgq[2 * p + 7])
                emit_wn_transpose2(p + 3)
            emit_pv(2 * p)
            emit_pv(2 * p + 1)
            emit_post_transposes(p)
            if p > 0:
                emit_out(p - 1)
            if p >= 3 and p % 2 == 1:
                emit_store((p - 3) // 2)
        emit_out(NP - 1)
        emit_store(2)
        emit_store(3)

        for pool in (work, psP, psB, psS, sing):
            pool.release()

    nc.compile()
    return nc


def _cos_sin():
    t = np.arange(T, dtype=np.float64)
    per = np.arange(1, NF + 1, dtype=np.float64)
    ang = 2 * math.pi * t[None, :] / per[:, None]
    return np.cos(ang).astype(np.float32), np.sin(ang).astype(np.float32)


def _d_patterns():
    d5 = np.empty((128, 3, WIN), np.float32)
    p = np.arange(128)[:, None]
    j = np.arange(WIN)[None, :]
    for k, off in enumerate((0, 64, 128)):
        d = np.abs(p + off - j).astype(np.float32)
        d[p + off == j] = DIAG_BIG
        d5[:, k, :] = d
    return d5


_COS, _SIN = _cos_sin()
_D5 = _d_patterns()


def _prep_core_inputs(inputs, b, hg):
    f32 = np.float32
    x_b = np.ascontiguousarray(np.asarray(inputs["x"])[b], dtype=f32)
    hsl = slice(hg * 256, (hg + 1) * 256)
    fsl = slice(hg * 8, (hg + 1) * 8)
    s = f32(1.0 / math.sqrt(128.0))
    cosT, sinT = _COS, _SIN

    Wq = np.asarray(inputs["W_query"], f32)[hsl] * s
    bq = np.asarray(inputs["b_query"], f32)[hsl] * s
    Wk = np.asarray(inputs["W_key"], f32)[hsl]
    bk = np.asarray(inputs["b_key"], f32)[hsl]
    Wc = np.asarray(inputs["W_content"], f32)[hsl]
    Wf = np.asarray(inputs["W_qfreq"], f32)[fsl] * f32(0.5)
    bf = np.asarray(inputs["b_qfreq"], f32)[fsl] * f32(0.5)
    Wd = np.asarray(inputs["W_qdecay"], f32)[fsl]
    bd = np.asarray(inputs["b_qdecay"], f32)[fsl]
    Wp = np.asarray(inputs["W_proj"], f32)
    Wp_hg = Wp[:, hg * 264:(hg + 1) * 264]

    # wfdT cols: [fq_h0 x2 (dup for cos/sin), fq_h1 x2, qd_h0, qd_h1]
    wfd = np.concatenate([Wf[0:4], Wf[0:4], Wf[4:8], Wf[4:8],
                          Wd[0:4], Wd[4:8]], axis=0)  # [24, 512]

    gco = np.zeros((8, 2), f32)
    dvec = -(np.arange(1, ND + 1, dtype=f32) / 8)
    gco[0:4, 0] = dvec
    gco[4:8, 1] = dvec

    wqkc = np.concatenate([Wq.T, Wk.T, Wc.T, wfd.T], axis=1)  # [512, 792]
    wp12 = np.stack([Wp_hg[:, 0:128].T, Wp_hg[:, 132:260].T])  # [2, 128, 512]

    # wp3cat rows: tsig proj (cos dup 0-3 + sin dup 4-7 per head) + bias row
    wp3cat = np.zeros((17, C), f32)
    wp3cat[0:8] = np.concatenate([Wp_hg[:, 128:132].T, Wp_hg[:, 128:132].T])
    wp3cat[8:16] = np.concatenate([Wp_hg[:, 260:264].T, Wp_hg[:, 260:264].T])
    if hg == 0:
        b_eff = np.asarray(inputs["b_proj"], f32).copy()
        bc = np.asarray(inputs["b_content"], f32)
        for h in range(HEADS):
            b_eff += Wp[:, 132 * h:132 * h + 128] @ bc[128 * h:128 * h + 128]
        wp3cat[16] = b_eff

    cs = np.concatenate([cosT, sinT], 0)  # [8, T]
    cs16 = np.concatenate([cs, cs], 0)    # [16, T]
    csT = np.ascontiguousarray(cs.T)      # [T, 8]
    # csT dup'd per head-aux: [128, tt, 2, 8]
    csTr = csT.reshape(16, 128, 8).transpose(1, 0, 2)  # [128, tt, 8]
    csT2 = np.concatenate([csTr[:, :, None, :], csTr[:, :, None, :]], axis=2)

    bigb = np.empty((128, XWCOLS), np.float32)
    bigb[:, XB0:WQ0] = x_b.reshape(4, 128, 2048).transpose(1, 0, 2).reshape(128, -1)
    bigb[:, WQ0:WP0] = wqkc.reshape(4, 128, 792).transpose(1, 0, 2).reshape(128, -1)
    bigb[:, WP0:CST0] = wp12.transpose(1, 0, 2).reshape(128, -1)
    bigb[:, CST0:XWCOLS] = csT2.reshape(128, -1)

    smallb = np.zeros((17, 6656), np.float32)
    smallb[0:16, 0:2048] = cs16
    smallb[0:8, 2048:4096] = cs
    smallb[8:16, 4096:6144] = cs
    smallb[:, 6144:6656] = wp3cat

    smallr = np.zeros((128, SRCOLS), f32)
    smallr[:, D50:D50 + 768] = _D5.reshape(128, -1)
    smallr[:, SR_I:SR_I + 128] = np.eye(128, dtype=f32)
    smallf = np.zeros((128, SFCOLS), f32)
    smallf[:, SF_BQ:SF_BQ + 2] = bq.reshape(2, 128).T
    smallf[:, SF_BK:SF_BK + 2] = bk.reshape(2, 128).T
    smallf[0:16, SF_FB] = np.concatenate([bf[0:4], bf[0:4], bf[4:8], bf[4:8]])
    smallf[0:8, SF_BD] = 0.5 * np.concatenate([bd[0:4], bd[4:8]])
    smallf[0:8, SF_GCO:SF_GCO + 2] = gco
    return {
        "bigb": bigb.astype(BF16),
        "smallb": smallb.astype(BF16),
        "smallf": smallf,
        "smallr": smallr,
    }


def get_nc():
    if "nc" not in _CACHE:
        _CACHE["nc"] = _build_nc()
    return _CACHE["nc"]


def make_in_maps(inputs):
    return [_prep_core_inputs(inputs, c // 2, c % 2) for c in range(8)]


def kernel(**inputs):
    from concourse.bass_utils import run_bass_kernel_spmd

    nc = get_nc()
    in_maps = make_in_maps(inputs)
    res = run_bass_kernel_spmd(nc, in_maps, core_ids=list(range(8)))
    x = np.asarray(inputs["x"], np.float32)
    out = np.empty((B, C, T), np.float32)
    for b in range(B):
        out[b] = res.results[2 * b]["out"].astype(np.float32)
        out[b] += res.results[2 * b + 1]["out"].astype(np.float32)
        out[b] += x[b]
    return out


# revision 31
# speedup vs baseline: 1.2007x; 1.0990x over previous
"""Trainium2 Bass kernel for nn_LocalState_1580547972191 (sparse_attention).

Contract: kernel(**inputs) takes FULL unsharded inputs (as from setup_inputs()),
returns FULL output [4, 512, 2048] f32. Internally shards across 8 NeuronCores:
core = (b, hg) with b = batch, hg = head-group (heads {2hg, 2hg+1}).

v2 design (cost-model-guided rewrite of the v1 baseline):
- Banded attention, 128-query blocks, 256-wide key window at 64-offset
  (w0 = clamp(128i-64, 0, 1792)); out-of-window softmax mass < 2e-8.
- Freq bias rank-2 folded into QK matmul via 8 augmented rows (cos/sin).
- Decay bias: one fused DVE/Pool scalar_tensor_tensor per (block, head):
  S += D * gneg, D = |t-s| pattern (3 patterns) with 1e4 on the diagonal
  (folds the eye-mask). gneg from a tanh-based sigmoid (tanh is in every
  activation table; avoids an act-table reload).
- fq/qd projections fused into ONE 24-row matmul per chunk (cost model:
  matmul cost ~ output free size only).
- exp without max-subtraction (logits bounded); one exp per block covers
  both heads ([128,2,256] PSUM spanning 2 banks).
- W^T via batched DMA transposes (4 blocks per instruction); zero-padded
  384-wide slabs keep PV contraction 128-aligned.
- PV computed transposed: omT[s,c] = sum_t W[t,s] content[c,t] with an
  fused aux rhs [csT(8) | ones | content(128)] so each series also yields
  time-sig partials AND sigma (the softmax denominator) for free (N=137).
- Normalization (x 1/sigma) applied per-partition on omT/oaT, then omT is
  DMA-transposed back into Res for the output projection.
- Output projection: content (2x K=128) + [tsig|bias] (K=17, ones row
  folds b_proj and W_proj@b_content) = 3 matmuls per (ot, n).
- bf16 partial outputs; host sums the two head-group partials + x.
- PE warmup matmuls bridge the p-state ramp across the initial DMA loads.
"""
import math
import sys

sys.path.insert(0, "/opt/trn_rl_repo")

import ml_dtypes
import numpy as np

HEADS, NF, ND = 4, 4, 4
B, C, T = 4, 512, 2048
NBLK, WIN = 16, 256
DIAG_BIG = 1.0e4
BF16 = ml_dtypes.bfloat16

_CACHE = {}

# bigb column layout (bf16, [128, XWCOLS])
XB0 = 0                  # xb [128, 4, 2048]
WQ0 = XB0 + 4 * 2048     # wqkc [128, 4, 792]
WP0 = WQ0 + 4 * 792      # wp12 [128, 2, 512]
CST0 = WP0 + 2 * 512     # csT dup [128, 16, 2, 8]
XWCOLS = CST0 + 16 * 16

# smallr column layout (f32r, [128, SRCOLS]): d5 + identity
D50 = 0
SR_I = D50 + 3 * 256
SRCOLS = SR_I + 128
# smallf column layout (f32, [128, SFCOLS])
SF_BQ = 0                # bq [128, 2] (scaled)
SF_BK = SF_BQ + 2        # bk [128, 2]
SF_FB = SF_BK + 2        # fbias16 [16, 1] at rows 0:16
SF_BD = SF_FB + 1        # 0.5*b_qdecay [8, 1] rows 0:8
SF_GCO = SF_BD + 1       # gco [8, 2] rows 0:8
SFCOLS = SF_GCO + 2

# CT layout: [auxA(9) | c_h0(128) | auxB(9) | c_h1(128)] = 274, pad 288
CT_A0, CT_C0, CT_A1, CT_C1, CTW = 0, 9, 137, 146, 288


def _w0_of_block(i):
    return min(max(128 * i - 64, 0), T - WIN)


def _off_of_block(i):
    # write offset of the active window inside the 384-wide padded slab
    return _w0_of_block(i) - 128 * (i - 1)


def _pat_of_block(i):
    return {0: 0, NBLK - 1: 2}.get(i, 1)


def _ks_of_block(i):
    return [k for k in range(3) if 0 <= i - 1 + k <= NBLK - 1]


def _TL(pool, shape, dtype, tag, **kw):
    return pool.tile(shape, dtype, name=tag, tag=tag, **kw)


def _build_nc():
    import concourse.mybir as mybir
    import concourse.tile as tile
    from concourse import bacc

    dt = mybir.dt
    f32, bf16 = dt.float32, dt.bfloat16
    Alu = mybir.AluOpType
    Act = mybir.ActivationFunctionType

    nc = bacc.Bacc("TRN2", target_bir_lowering=False, debug=False, num_devices=8)

    bigb_d = nc.dram_tensor("bigb", [128, XWCOLS], bf16, kind="ExternalInput")
    smallb_d = nc.dram_tensor("smallb", [17, 6656], bf16, kind="ExternalInput")
    smallf_d = nc.dram_tensor("smallf", [128, SFCOLS], f32, kind="ExternalInput")
    smallr_d = nc.dram_tensor("smallr", [128, SRCOLS], dt.float32r, kind="ExternalInput")
    out_d = nc.dram_tensor("out", [C, T], bf16, kind="ExternalOutput")

    with tile.TileContext(nc) as tc:
        sing = tc.alloc_tile_pool(name="sing", bufs=1)
        psS = tc.alloc_tile_pool(name="psS", bufs=2, space="PSUM")
        psB = tc.alloc_tile_pool(name="psB", bufs=2, space="PSUM")
        psP = tc.alloc_tile_pool(name="psP", bufs=2, space="PSUM")
        work = tc.alloc_tile_pool(name="work", bufs=4)

        # ---- persistent SBUF tiles ----
        XW = _TL(sing, [128, XWCOLS], bf16, tag="XW")
        CS = _TL(sing, [17, 6656], bf16, tag="CS")
        SF = _TL(sing, [128, SFCOLS], f32, tag="SF")
        SR = _TL(sing, [128, SRCOLS], dt.float32r, tag="SR")
        Q_sb = [_TL(sing, [128, T], bf16, tag=f"Q{h}") for h in range(2)]
        K_sb = [_TL(sing, [128, T], bf16, tag=f"K{h}") for h in range(2)]
        CT = _TL(sing, [128, NBLK, CTW], bf16, tag="CT")
        fqh16 = _TL(sing, [16, T], bf16, tag="fqh16")
        Qaux16 = _TL(sing, [16, T], bf16, tag="Qaux16")
        sig_sb = _TL(sing, [8, T], f32, tag="sig")
        gneg_sb = _TL(sing, [128, 2 * NBLK], f32, tag="gneg")
        recip_sb = _TL(sing, [128, 2 * NBLK], f32, tag="recip")
        wn2 = _TL(sing, [128, NBLK, 2, 384], bf16, tag="wn2")
        WnTu = _TL(sing, [128, NBLK, 2, 3, 128], bf16, tag="WnTu")
        OMPT = _TL(sing, [128, NBLK, 3, 128], bf16, tag="OMPT")
        RES3 = _TL(sing, [128, NBLK, 3, 128], bf16, tag="RES3")
        OB = _TL(sing, [128, 4, T], bf16, tag="OB")
        warm = _TL(sing, [128, 512], bf16, tag="warm")

        xb = [XW[:, XB0 + 2048 * k:XB0 + 2048 * (k + 1)] for k in range(4)]
        wqkc = [XW[:, WQ0 + 792 * k:WQ0 + 792 * (k + 1)] for k in range(4)]
        wqT = [w[:, 0:256] for w in wqkc]
        wkT = [w[:, 256:512] for w in wqkc]
        wcT = [w[:, 512:768] for w in wqkc]
        wfdT = [w[:, 768:792] for w in wqkc]
        wp12 = [XW[:, WP0 + 512 * h:WP0 + 512 * (h + 1)] for h in range(2)]
        cs16 = CS[0:16, 0:2048]
        csz = [CS[0:16, 2048:4096], CS[0:16, 4096:6144]]
        wp3cat = CS[0:17, 6144:6656]
        d5 = [SR[:, D50 + 256 * p:D50 + 256 * (p + 1)] for p in range(3)]
        bq = SF[:, SF_BQ:SF_BQ + 2]
        bk = SF[:, SF_BK:SF_BK + 2]
        fb16 = SF[0:16, SF_FB:SF_FB + 1]
        bdh = SF[0:8, SF_BD:SF_BD + 1]
        gco = SF[0:8, SF_GCO:SF_GCO + 2]
        I_sb = SR[:, SR_I:SR_I + 128]

        # ---- init: memsets (overlap the loads) + PE warmup ----
        nc.vector.memset(warm[:, :], 0.0)
        nc.gpsimd.memset(wn2[:, :, :, 0:64], 0.0)
        nc.gpsimd.memset(wn2[:, :, :, 320:384], 0.0)
        nc.vector.memset(wn2[:, 0, :, 64:128], 0.0)
        nc.vector.memset(wn2[:, NBLK - 1, :, 256:320], 0.0)
        nc.gpsimd.memset(CT[:, :, CT_A0 + 8:CT_A0 + 9], 1.0)
        nc.gpsimd.memset(CT[:, :, CT_A1 + 8:CT_A1 + 9], 1.0)
        nc.vector.memset(OMPT[:, :, 2, 16:17], 1.0)

        # warmup matmuls: keep PE continuously busy through the input DMAs so
        # the p-state is fully ramped when real work starts.
        for w in range(18):
            pw = _TL(psP, [128, 512], f32, tag="pP", padded_shape=[128, 512])
            nc.tensor.matmul(pw, warm[:, 0:128], warm, start=True, stop=True)

        # ---- loads (Q/K weights and first xb chunk first) ----
        wsrc = bigb_d.ap()[:, WQ0:WP0].rearrange("p (k c) -> p k c", k=4)
        wdst = XW[:, WQ0:WP0].rearrange("p (k c) -> p k c", k=4)
        nc.sync.dma_start(out=wdst[:, :, 0:512], in_=wsrc[:, :, 0:512])
        xbsrc = bigb_d.ap()[:, XB0:WQ0].rearrange("p (k t) -> p k t", k=4)
        xbdst = XW[:, XB0:WQ0].rearrange("p (k t) -> p k t", k=4)
        nc.sync.dma_start(out=xbdst[:, :, 0:512], in_=xbsrc[:, :, 0:512])
        nc.sync.dma_start(out=SF, in_=smallf_d[:, :])
        nc.sync.dma_start(out=SR, in_=smallr_d[:, :])
        nc.sync.dma_start(out=wdst[:, :, 512:792], in_=wsrc[:, :, 512:792])
        nc.sync.dma_start(out=XW[:, WP0:XWCOLS], in_=bigb_d[:, WP0:XWCOLS])
        nc.sync.dma_start(out=CS, in_=smallb_d[:, :])
        for n in range(1, 4):
            nc.sync.dma_start(out=xbdst[:, :, 512 * n:512 * (n + 1)],
                              in_=xbsrc[:, :, 512 * n:512 * (n + 1)])

        csT_src = XW[:, CST0:CST0 + 256].rearrange("p (tt c) -> p tt c", tt=16)
        nc.vector.tensor_copy(out=CT[:, :, CT_A0:CT_A0 + 8], in_=csT_src[:, :, 0:8])
        nc.gpsimd.tensor_copy(out=CT[:, :, CT_A1:CT_A1 + 8], in_=csT_src[:, :, 8:16])

        # ---- phase A pieces ----
        def emit_qk(n):
            cols = slice(512 * n, 512 * (n + 1))
            for h in range(2):
                pq = _TL(psP, [128, 512], f32, tag="pP", padded_shape=[128, 512])
                for k in range(4):
                    nc.tensor.matmul(pq, wqT[k][:, 128 * h:128 * (h + 1)],
                                     xb[k][:, cols], start=(k == 0), stop=(k == 3))
                if h == 0:
                    nc.scalar.activation(out=Q_sb[h][:, cols], in_=pq,
                                         func=Act.Identity, bias=bq[:, h:h + 1])
                else:
                    nc.vector.tensor_scalar(out=Q_sb[h][:, cols], in0=pq,
                                            scalar1=bq[:, h:h + 1], scalar2=None,
                                            op0=Alu.add)
                pk = _TL(psP, [128, 512], f32, tag="pP", padded_shape=[128, 512])
                for k in range(4):
                    nc.tensor.matmul(pk, wkT[k][:, 128 * h:128 * (h + 1)],
                                     xb[k][:, cols], start=(k == 0), stop=(k == 3))
                if h == 1:
                    nc.scalar.activation(out=K_sb[h][:, cols], in_=pk,
                                         func=Act.Identity, bias=bk[:, h:h + 1])
                else:
                    nc.vector.tensor_scalar(out=K_sb[h][:, cols], in0=pk,
                                            scalar1=bk[:, h:h + 1], scalar2=None,
                                            op0=Alu.add)

        def emit_fd(n):
            cols = slice(512 * n, 512 * (n + 1))
            pfq = _TL(psB, [16, 512], f32, tag="pB", padded_shape=[128, 512])
            for k in range(4):
                nc.tensor.matmul(pfq, wfdT[k][:, 0:16], xb[k][:, cols],
                                 start=(k == 0), stop=(k == 3))
            nc.scalar.activation(out=fqh16[:, cols], in_=pfq,
                                 func=Act.Identity, bias=fb16)
            pqd = _TL(psB, [8, 512], f32, tag="pB", padded_shape=[128, 512])
            for k in range(4):
                nc.tensor.matmul(pqd, wfdT[k][:, 16:24], xb[k][:, cols],
                                 start=(k == 0), stop=(k == 3))
            nc.scalar.activation(out=sig_sb[:, cols], in_=pqd,
                                 func=Act.Tanh, bias=bdh, scale=0.5)
            nc.vector.tensor_tensor(out=Qaux16[:, cols], in0=cs16[:, cols],
                                    in1=fqh16[:, cols], op=Alu.mult)

        def emit_ct(tt, j):
            rows = slice(128 * tt, 128 * (tt + 1))
            pc = _TL(psP, [128, 256], f32, tag="pP", padded_shape=[128, 512])
            for k in range(4):
                nc.tensor.matmul(pc, xb[k][:, rows], wcT[k],
                                 start=(k == 0), stop=(k == 3))
            if j % 2 == 0:
                nc.vector.tensor_copy(out=CT[:, tt, CT_C0:CT_C0 + 128], in_=pc[:, 0:128])
                nc.vector.tensor_copy(out=CT[:, tt, CT_C1:CT_C1 + 128], in_=pc[:, 128:256])
            else:
                nc.scalar.activation(out=CT[:, tt, CT_C0:CT_C0 + 128],
                                     in_=pc[:, 0:128], func=Act.Copy)
                nc.scalar.activation(out=CT[:, tt, CT_C1:CT_C1 + 128],
                                     in_=pc[:, 128:256], func=Act.Copy)

        def emit_gneg(i):
            pg = _TL(psB, [128, 2], f32, tag="pB", padded_shape=[128, 512])
            nc.tensor.matmul(pg, sig_sb[:, 128 * i:128 * (i + 1)], gco,
                             start=True, stop=True)
            nc.vector.tensor_scalar(out=gneg_sb[:, 2 * i:2 * i + 2], in0=pg,
                                    scalar1=-1.25, scalar2=None, op0=Alu.add)

        # ---- phase B: attention S + exp for one block ----
        def emit_diag(i):
            dg = _TL(work, [128, 2, 128], dt.float32r, tag="diag")
            for h in range(2):
                nc.gpsimd.tensor_scalar(out=dg[:, h, :], in0=I_sb,
                                        scalar1=gneg_sb[:, 2 * i + h:2 * i + h + 1],
                                        scalar2=None, op0=Alu.mult)
            return dg

        def emit_S(i, dg):
            s0, w0 = 128 * i, _w0_of_block(i)
            off = _off_of_block(i)
            s2t = _TL(psS, [128, 2, 512], f32, tag="pS", padded_shape=[128, 2, 512])
            for h in range(2):
                nc.tensor.matmul(s2t[:, h, 0:WIN], Q_sb[h][:, s0:s0 + 128],
                                 K_sb[h][:, w0:w0 + WIN], start=True, stop=False)
                nc.tensor.matmul(s2t[:, h, 0:WIN], Qaux16[:, s0:s0 + 128],
                                 csz[h][:, w0:w0 + WIN], start=False, stop=False)
                nc.tensor.matmul(s2t[:, h, 0:WIN], dg[:, h, :],
                                 d5[_pat_of_block(i)], start=False, stop=True)
            nc.scalar.activation(out=wn2[:, i, :, off:off + WIN],
                                 in_=s2t[:, :, 0:WIN], func=Act.Exp)

        # ---- phase B: PV (transposed) for one block ----
        def emit_pv(i):
            ks = _ks_of_block(i)
            pomh = []
            for h in range(2):
                pom = _TL(psB, [128, 137], f32, tag="pB", padded_shape=[128, 512])
                pomh.append(pom)
                rhs0 = CT_A0 if h == 0 else CT_A1
                for j, k in enumerate(ks):
                    nc.tensor.matmul(
                        pom, WnTu[:, i, h, k, :],
                        CT[:, i - 1 + k, rhs0:rhs0 + 137],
                        start=(j == 0), stop=(j == len(ks) - 1))
            for h in range(2):
                nc.vector.reciprocal(out=recip_sb[:, 2 * i + h:2 * i + h + 1],
                                     in_=pomh[h][:, 8:9])
            nc.vector.tensor_scalar(out=OMPT[:, i, 0, :], in0=pomh[0][:, 9:137],
                                     scalar1=recip_sb[:, 2 * i:2 * i + 1],
                                     scalar2=None, op0=Alu.mult)
            nc.scalar.activation(out=OMPT[:, i, 1, :], in_=pomh[1][:, 9:137],
                                 func=Act.Identity,
                                 scale=recip_sb[:, 2 * i + 1:2 * i + 2])
            for h in range(2):
                nc.vector.scalar_tensor_tensor(
                    out=OMPT[:, i, 2, 8 * h:8 * h + 8], in0=pomh[h][:, 0:8],
                    scalar=recip_sb[:, 2 * i + h:2 * i + h + 1],
                    in1=CT[:, i, CT_A0:CT_A0 + 8], op0=Alu.mult, op1=Alu.mult)

        def emit_wn_transpose1(i):
            nc.sync.dma_start_transpose(
                out=WnTu[:, i, :, :, :].rearrange("p hh k e -> p (hh k) e"),
                in_=wn2[:, i, :, :])

        def emit_post_transposes(p):
            nc.sync.dma_start_transpose(
                out=RES3[:, 2 * p:2 * p + 2, :, :],
                in_=OMPT[:, 2 * p:2 * p + 2, :, :])

        # ---- phase C: output projection for block pair p ----
        def emit_out(p):
            cols = slice(256 * p, 256 * (p + 1))
            for ot in range(4):
                osl = slice(128 * ot, 128 * (ot + 1))
                pp = _TL(psP, [128, 256], f32, tag="pP", padded_shape=[128, 512])
                nc.tensor.matmul(pp, wp12[0][:, osl], RES3[:, 2 * p:2 * p + 2, 0, :],
                                 start=True, stop=False)
                nc.tensor.matmul(pp, wp12[1][:, osl], RES3[:, 2 * p:2 * p + 2, 1, :],
                                 start=False, stop=False)
                nc.tensor.matmul(pp, wp3cat[:, osl],
                                 RES3[0:17, 2 * p:2 * p + 2, 2, :],
                                 start=False, stop=True)
                if ot % 2 == 0:
                    nc.vector.tensor_copy(out=OB[:, ot, cols], in_=pp)
                else:
                    nc.scalar.activation(out=OB[:, ot, cols], in_=pp, func=Act.Copy)

        def emit_store(g):
            cs_ = slice(512 * g, 512 * (g + 1))
            nc.scalar.dma_start(
                out=out_d.ap().rearrange("(ot p) t -> p ot t", p=128)[:, :, cs_],
                in_=OB[:, :, cs_])

        def emit_store_pair(p):
            cs_ = slice(256 * p, 256 * (p + 1))
            nc.scalar.dma_start(
                out=out_d.ap().rearrange("(ot p) t -> p ot t", p=128)[:, :, cs_],
                in_=OB[:, :, cs_])

        NP = NBLK // 2
        # prologue
        emit_qk(0)
        emit_fd(0)
        emit_ct(0, 0)
        emit_ct(1, 1)
        for i in range(4):
            emit_gneg(i)
        emit_qk(1)
        emit_fd(1)
        emit_ct(2, 2)
        emit_ct(3, 3)
        for i in range(4, 8):
            emit_gneg(i)
        dgq = [emit_diag(0), emit_diag(1)]
        for q in range(3):
            dgq.append(emit_diag(2 * q + 2))
            dgq.append(emit_diag(2 * q + 3))
            emit_S(2 * q, dgq[2 * q])
            emit_wn_transpose1(2 * q)
            emit_S(2 * q + 1, dgq[2 * q + 1])
            emit_wn_transpose1(2 * q + 1)
        for p in range(NP):
            if p == 0:
                emit_qk(2)
                emit_fd(2)
                for i in range(8, 12):
                    emit_gneg(i)
            if p == 2:
                emit_qk(3)
                emit_fd(3)
                for i in range(12, 16):
                    emit_gneg(i)
            if p < 6:
                emit_ct(2 * p + 4, 2 * p)
                emit_ct(2 * p + 5, 2 * p + 1)
            if p + 3 < NP:
                if 2 * p + 8 < NBLK:
                    dgq.append(emit_diag(2 * p + 8))
                if 2 * p + 9 < NBLK:
                    dgq.append(emit_diag(2 * p + 9))
                emit_S(2 * p + 6, dgq[2 * p + 6])
                emit_wn_transpose1(2 * p + 6)
                emit_S(2 * p + 7, dgq[2 * p + 7])
                emit_wn_transpose1(2 * p + 7)
            if p > 0:
                emit_out(p - 1)
            emit_pv(2 * p)
            emit_pv(2 * p + 1)
            emit_post_transposes(p)
            if p >= 3 and p % 2 == 1:
                emit_store((p - 3) // 2)
        emit_out(NP - 1)
        emit_store_pair(6)
        emit_store_pair(7)

        for pool in (work, psP, psB, psS, sing):
            pool.release()

    nc.compile()
    return nc


def _cos_sin():
    t = np.arange(T, dtype=np.float64)
    per = np.arange(1, NF + 1, dtype=np.float64)
    ang = 2 * math.pi * t[None, :] / per[:, None]
    return np.cos(ang).astype(np.float32), np.sin(ang).astype(np.float32)


def _d_patterns():
    d5 = np.empty((128, 3, WIN), np.float32)
    p = np.arange(128)[:, None]
    j = np.arange(WIN)[None, :]
    for k, off in enumerate((0, 64, 128)):
        d = np.abs(p + off - j).astype(np.float32)
        d[p + off == j] = DIAG_BIG
        d5[:, k, :] = d
    return d5


_COS, _SIN = _cos_sin()
_D5 = _d_patterns()


def _prep_core_inputs(inputs, b, hg):
    f32 = np.float32
    x_b = np.ascontiguousarray(np.asarray(inputs["x"])[b], dtype=f32)
    hsl = slice(hg * 256, (hg + 1) * 256)
    fsl = slice(hg * 8, (hg + 1) * 8)
    s = f32(1.0 / math.sqrt(128.0))
    cosT, sinT = _COS, _SIN

    Wq = np.asarray(inputs["W_query"], f32)[hsl] * s
    bq = np.asarray(inputs["b_query"], f32)[hsl] * s
    Wk = np.asarray(inputs["W_key"], f32)[hsl]
    bk = np.asarray(inputs["b_key"], f32)[hsl]
    Wc = np.asarray(inputs["W_content"], f32)[hsl]
    Wf = np.asarray(inputs["W_qfreq"], f32)[fsl] * f32(0.5)
    bf = np.asarray(inputs["b_qfreq"], f32)[fsl] * f32(0.5)
    Wd = np.asarray(inputs["W_qdecay"], f32)[fsl]
    bd = np.asarray(inputs["b_qdecay"], f32)[fsl]
    Wp = np.asarray(inputs["W_proj"], f32)
    Wp_hg = Wp[:, hg * 264:(hg + 1) * 264]

    # wfdT cols: [fq_h0 x2 (dup for cos/sin), fq_h1 x2, qd_h0, qd_h1]
    wfd = np.concatenate([Wf[0:4], Wf[0:4], Wf[4:8], Wf[4:8],
                          Wd[0:4], Wd[4:8]], axis=0)  # [24, 512]

    gco = np.zeros((8, 2), f32)
    dvec = -(np.arange(1, ND + 1, dtype=f32) / 8)
    gco[0:4, 0] = dvec
    gco[4:8, 1] = dvec

    wqkc = np.concatenate([Wq.T, Wk.T, Wc.T, wfd.T], axis=1)  # [512, 792]
    wp12 = np.stack([Wp_hg[:, 0:128].T, Wp_hg[:, 132:260].T])  # [2, 128, 512]

    # wp3cat rows: tsig proj (cos dup 0-3 + sin dup 4-7 per head) + bias row
    wp3cat = np.zeros((17, C), f32)
    wp3cat[0:8] = np.concatenate([Wp_hg[:, 128:132].T, Wp_hg[:, 128:132].T])
    wp3cat[8:16] = np.concatenate([Wp_hg[:, 260:264].T, Wp_hg[:, 260:264].T])
    if hg == 0:
        b_eff = np.asarray(inputs["b_proj"], f32).copy()
        bc = np.asarray(inputs["b_content"], f32)
        for h in range(HEADS):
            b_eff += Wp[:, 132 * h:132 * h + 128] @ bc[128 * h:128 * h + 128]
        wp3cat[16] = b_eff

    cs = np.concatenate([cosT, sinT], 0)  # [8, T]
    cs16 = np.concatenate([cs, cs], 0)    # [16, T]
    csT = np.ascontiguousarray(cs.T)      # [T, 8]
    # csT dup'd per head-aux: [128, tt, 2, 8]
    csTr = csT.reshape(16, 128, 8).transpose(1, 0, 2)  # [128, tt, 8]
    csT2 = np.concatenate([csTr[:, :, None, :], csTr[:, :, None, :]], axis=2)

    bigb = np.empty((128, XWCOLS), np.float32)
    bigb[:, XB0:WQ0] = x_b.reshape(4, 128, 2048).transpose(1, 0, 2).reshape(128, -1)
    bigb[:, WQ0:WP0] = wqkc.reshape(4, 128, 792).transpose(1, 0, 2).reshape(128, -1)
    bigb[:, WP0:CST0] = wp12.transpose(1, 0, 2).reshape(128, -1)
    bigb[:, CST0:XWCOLS] = csT2.reshape(128, -1)

    smallb = np.zeros((17, 6656), np.float32)
    smallb[0:16, 0:2048] = cs16
    smallb[0:8, 2048:4096] = cs
    smallb[8:16, 4096:6144] = cs
    smallb[:, 6144:6656] = wp3cat

    smallr = np.zeros((128, SRCOLS), f32)
    smallr[:, D50:D50 + 768] = _D5.reshape(128, -1)
    smallr[:, SR_I:SR_I + 128] = np.eye(128, dtype=f32)
    smallf = np.zeros((128, SFCOLS), f32)
    smallf[:, SF_BQ:SF_BQ + 2] = bq.reshape(2, 128).T
    smallf[:, SF_BK:SF_BK + 2] = bk.reshape(2, 128).T
    smallf[0:16, SF_FB] = np.concatenate([bf[0:4], bf[0:4], bf[4:8], bf[4:8]])
    smallf[0:8, SF_BD] = 0.5 * np.concatenate([bd[0:4], bd[4:8]])
    smallf[0:8, SF_GCO:SF_GCO + 2] = gco
    return {
        "bigb": bigb.astype(BF16),
        "smallb": smallb.astype(BF16),
        "smallf": smallf,
        "smallr": smallr,
    }


def get_nc():
    if "nc" not in _CACHE:
        _CACHE["nc"] = _build_nc()
    return _CACHE["nc"]


def make_in_maps(inputs):
    return [_prep_core_inputs(inputs, c // 2, c % 2) for c in range(8)]


def kernel(**inputs):
    from concourse.bass_utils import run_bass_kernel_spmd

    nc = get_nc()
    in_maps = make_in_maps(inputs)
    res = run_bass_kernel_spmd(nc, in_maps, core_ids=list(range(8)))
    x = np.asarray(inputs["x"], np.float32)
    out = np.empty((B, C, T), np.float32)
    for b in range(B):
        out[b] = res.results[2 * b]["out"].astype(np.float32)
        out[b] += res.results[2 * b + 1]["out"].astype(np.float32)
        out[b] += x[b]
    return out


# revision 33
# speedup vs baseline: 1.2494x; 1.0405x over previous
"""Trainium2 Bass kernel for nn_LocalState_1580547972191 (sparse_attention).

Contract: kernel(**inputs) takes FULL unsharded inputs (as from setup_inputs()),
returns FULL output [4, 512, 2048] f32. Internally shards across 8 NeuronCores:
core = (b, hg) with b = batch, hg = head-group (heads {2hg, 2hg+1}).

v2 design (cost-model-guided rewrite of the v1 baseline):
- Banded attention, 128-query blocks, 256-wide key window at 64-offset
  (w0 = clamp(128i-64, 0, 1792)); out-of-window softmax mass < 2e-8.
- Freq bias rank-2 folded into QK matmul via 8 augmented rows (cos/sin).
- Decay bias: one fused DVE/Pool scalar_tensor_tensor per (block, head):
  S += D * gneg, D = |t-s| pattern (3 patterns) with 1e4 on the diagonal
  (folds the eye-mask). gneg from a tanh-based sigmoid (tanh is in every
  activation table; avoids an act-table reload).
- fq/qd projections fused into ONE 24-row matmul per chunk (cost model:
  matmul cost ~ output free size only).
- exp without max-subtraction (logits bounded); one exp per block covers
  both heads ([128,2,256] PSUM spanning 2 banks).
- W^T via batched DMA transposes (4 blocks per instruction); zero-padded
  384-wide slabs keep PV contraction 128-aligned.
- PV computed transposed: omT[s,c] = sum_t W[t,s] content[c,t] with an
  fused aux rhs [csT(8) | ones | content(128)] so each series also yields
  time-sig partials AND sigma (the softmax denominator) for free (N=137).
- Normalization (x 1/sigma) applied per-partition on omT/oaT, then omT is
  DMA-transposed back into Res for the output projection.
- Output projection: content (2x K=128) + [tsig|bias] (K=17, ones row
  folds b_proj and W_proj@b_content) = 3 matmuls per (ot, n).
- bf16 partial outputs; host sums the two head-group partials + x.
- PE warmup matmuls bridge the p-state ramp across the initial DMA loads.
"""
import math
import sys

sys.path.insert(0, "/opt/trn_rl_repo")

import ml_dtypes
import numpy as np

HEADS, NF, ND = 4, 4, 4
B, C, T = 4, 512, 2048
NBLK, WIN = 16, 256
DIAG_BIG = 1.0e4
BF16 = ml_dtypes.bfloat16

_CACHE = {}

# bigb column layout (bf16, [128, XWCOLS])
XB0 = 0                  # xb [128, 4, 2048]
WQ0 = XB0 + 4 * 2048     # wqkc [128, 4, 792]
WP0 = WQ0 + 4 * 792      # wp12 [128, 2, 512]
CST0 = WP0 + 2 * 512     # csT dup [128, 16, 2, 8]
XWCOLS = CST0 + 16 * 16

# smallr column layout (f32r, [128, SRCOLS]): d5 + identity
D50 = 0
SR_I = D50 + 3 * 256
SRCOLS = SR_I + 128
# smallf column layout (f32, [128, SFCOLS])
SF_BQ = 0                # bq [128, 2] (scaled)
SF_BK = SF_BQ + 2        # bk [128, 2]
SF_FB = SF_BK + 2        # fbias16 [16, 1] at rows 0:16
SF_BD = SF_FB + 1        # 0.5*b_qdecay [8, 1] rows 0:8
SF_GCO = SF_BD + 1       # gco [8, 2] rows 0:8
SFCOLS = SF_GCO + 2

# CT layout: [auxA(9) | c_h0(128) | auxB(9) | c_h1(128)] = 274, pad 288
CT_A0, CT_C0, CT_A1, CT_C1, CTW = 0, 9, 137, 146, 288


def _w0_of_block(i):
    return min(max(128 * i - 64, 0), T - WIN)


def _off_of_block(i):
    # write offset of the active window inside the 384-wide padded slab
    return _w0_of_block(i) - 128 * (i - 1)


def _pat_of_block(i):
    return {0: 0, NBLK - 1: 2}.get(i, 1)


def _ks_of_block(i):
    return [k for k in range(3) if 0 <= i - 1 + k <= NBLK - 1]


def _TL(pool, shape, dtype, tag, **kw):
    return pool.tile(shape, dtype, name=tag, tag=tag, **kw)


def _build_nc():
    import concourse.mybir as mybir
    import concourse.tile as tile
    from concourse import bacc

    dt = mybir.dt
    f32, bf16 = dt.float32, dt.bfloat16
    Alu = mybir.AluOpType
    Act = mybir.ActivationFunctionType

    nc = bacc.Bacc("TRN2", target_bir_lowering=False, debug=False, num_devices=8)

    bigb_d = nc.dram_tensor("bigb", [128, XWCOLS], bf16, kind="ExternalInput")
    smallb_d = nc.dram_tensor("smallb", [17, 6656], bf16, kind="ExternalInput")
    smallf_d = nc.dram_tensor("smallf", [128, SFCOLS], f32, kind="ExternalInput")
    smallr_d = nc.dram_tensor("smallr", [128, SRCOLS], dt.float32r, kind="ExternalInput")
    out_d = nc.dram_tensor("out", [C, T], bf16, kind="ExternalOutput")

    with tile.TileContext(nc) as tc:
        sing = tc.alloc_tile_pool(name="sing", bufs=1)
        psS = tc.alloc_tile_pool(name="psS", bufs=2, space="PSUM")
        psB = tc.alloc_tile_pool(name="psB", bufs=2, space="PSUM")
        psP = tc.alloc_tile_pool(name="psP", bufs=2, space="PSUM")
        work = tc.alloc_tile_pool(name="work", bufs=4)

        # ---- persistent SBUF tiles ----
        XW = _TL(sing, [128, XWCOLS], bf16, tag="XW")
        CS = _TL(sing, [17, 6656], bf16, tag="CS")
        SF = _TL(sing, [128, SFCOLS], f32, tag="SF")
        SR = _TL(sing, [128, SRCOLS], dt.float32r, tag="SR")
        Q_sb = [_TL(sing, [128, T], bf16, tag=f"Q{h}") for h in range(2)]
        K_sb = [_TL(sing, [128, T], bf16, tag=f"K{h}") for h in range(2)]
        CT = _TL(sing, [128, NBLK, CTW], bf16, tag="CT")
        fqh16 = _TL(sing, [16, T], bf16, tag="fqh16")
        Qaux16 = _TL(sing, [16, T], bf16, tag="Qaux16")
        sig_sb = _TL(sing, [8, T], f32, tag="sig")
        gneg_sb = _TL(sing, [128, 2 * NBLK], f32, tag="gneg")
        recip_sb = _TL(sing, [128, 2 * NBLK], f32, tag="recip")
        wn2 = _TL(sing, [128, NBLK, 2, 384], bf16, tag="wn2")
        WnTu = _TL(sing, [128, NBLK, 2, 3, 128], bf16, tag="WnTu")
        OMPT = _TL(sing, [128, NBLK, 3, 128], bf16, tag="OMPT")
        RES3 = _TL(sing, [128, NBLK, 3, 128], bf16, tag="RES3")
        OB = _TL(sing, [128, 4, T], bf16, tag="OB")
        warm = _TL(sing, [128, 512], bf16, tag="warm")

        xb = [XW[:, XB0 + 2048 * k:XB0 + 2048 * (k + 1)] for k in range(4)]
        wqkc = [XW[:, WQ0 + 792 * k:WQ0 + 792 * (k + 1)] for k in range(4)]
        wqT = [w[:, 0:256] for w in wqkc]
        wkT = [w[:, 256:512] for w in wqkc]
        wcT = [w[:, 512:768] for w in wqkc]
        wfdT = [w[:, 768:792] for w in wqkc]
        wp12 = [XW[:, WP0 + 512 * h:WP0 + 512 * (h + 1)] for h in range(2)]
        cs16 = CS[0:16, 0:2048]
        csz = [CS[0:16, 2048:4096], CS[0:16, 4096:6144]]
        wp3cat = CS[0:17, 6144:6656]
        d5 = [SR[:, D50 + 256 * p:D50 + 256 * (p + 1)] for p in range(3)]
        bq = SF[:, SF_BQ:SF_BQ + 2]
        bk = SF[:, SF_BK:SF_BK + 2]
        fb16 = SF[0:16, SF_FB:SF_FB + 1]
        bdh = SF[0:8, SF_BD:SF_BD + 1]
        gco = SF[0:8, SF_GCO:SF_GCO + 2]
        I_sb = SR[:, SR_I:SR_I + 128]

        # ---- init: memsets (overlap the loads) + PE warmup ----
        nc.vector.memset(warm[:, :], 0.0)
        nc.gpsimd.memset(wn2[:, :, :, 0:64], 0.0)
        nc.gpsimd.memset(wn2[:, :, :, 320:384], 0.0)
        nc.vector.memset(wn2[:, 0, :, 64:128], 0.0)
        nc.vector.memset(wn2[:, NBLK - 1, :, 256:320], 0.0)
        nc.gpsimd.memset(CT[:, :, CT_A0 + 8:CT_A0 + 9], 1.0)
        nc.gpsimd.memset(CT[:, :, CT_A1 + 8:CT_A1 + 9], 1.0)
        nc.vector.memset(OMPT[:, :, 2, 16:17], 1.0)

        # warmup matmuls: keep PE continuously busy through the input DMAs so
        # the p-state is fully ramped when real work starts.
        for w in range(18):
            pw = _TL(psP, [128, 512], f32, tag="pP", padded_shape=[128, 512])
            nc.tensor.matmul(pw, warm[:, 0:128], warm, start=True, stop=True)

        # ---- loads (Q/K weights and first xb chunk first) ----
        wsrc = bigb_d.ap()[:, WQ0:WP0].rearrange("p (k c) -> p k c", k=4)
        wdst = XW[:, WQ0:WP0].rearrange("p (k c) -> p k c", k=4)
        nc.sync.dma_start(out=wdst[:, :, 0:512], in_=wsrc[:, :, 0:512])
        xbsrc = bigb_d.ap()[:, XB0:WQ0].rearrange("p (k t) -> p k t", k=4)
        xbdst = XW[:, XB0:WQ0].rearrange("p (k t) -> p k t", k=4)
        nc.sync.dma_start(out=xbdst[:, :, 0:512], in_=xbsrc[:, :, 0:512])
        nc.sync.dma_start(out=SF, in_=smallf_d[:, :])
        nc.sync.dma_start(out=SR, in_=smallr_d[:, :])
        nc.sync.dma_start(out=wdst[:, :, 512:792], in_=wsrc[:, :, 512:792])
        nc.sync.dma_start(out=XW[:, WP0:XWCOLS], in_=bigb_d[:, WP0:XWCOLS])
        nc.sync.dma_start(out=CS, in_=smallb_d[:, :])
        for n in range(1, 4):
            nc.sync.dma_start(out=xbdst[:, :, 512 * n:512 * (n + 1)],
                              in_=xbsrc[:, :, 512 * n:512 * (n + 1)])

        csT_src = XW[:, CST0:CST0 + 256].rearrange("p (tt c) -> p tt c", tt=16)
        nc.vector.tensor_copy(out=CT[:, :, CT_A0:CT_A0 + 8], in_=csT_src[:, :, 0:8])
        nc.gpsimd.tensor_copy(out=CT[:, :, CT_A1:CT_A1 + 8], in_=csT_src[:, :, 8:16])

        # ---- phase A pieces ----
        def emit_qk(n):
            cols = slice(512 * n, 512 * (n + 1))
            for h in range(2):
                pq = _TL(psP, [128, 512], f32, tag="pP", padded_shape=[128, 512])
                for k in range(4):
                    nc.tensor.matmul(pq, wqT[k][:, 128 * h:128 * (h + 1)],
                                     xb[k][:, cols], start=(k == 0), stop=(k == 3))
                if h == 0:
                    nc.scalar.activation(out=Q_sb[h][:, cols], in_=pq,
                                         func=Act.Identity, bias=bq[:, h:h + 1])
                else:
                    nc.vector.tensor_scalar(out=Q_sb[h][:, cols], in0=pq,
                                            scalar1=bq[:, h:h + 1], scalar2=None,
                                            op0=Alu.add)
                pk = _TL(psP, [128, 512], f32, tag="pP", padded_shape=[128, 512])
                for k in range(4):
                    nc.tensor.matmul(pk, wkT[k][:, 128 * h:128 * (h + 1)],
                                     xb[k][:, cols], start=(k == 0), stop=(k == 3))
                if h == 1:
                    nc.scalar.activation(out=K_sb[h][:, cols], in_=pk,
                                         func=Act.Identity, bias=bk[:, h:h + 1])
                else:
                    nc.vector.tensor_scalar(out=K_sb[h][:, cols], in0=pk,
                                            scalar1=bk[:, h:h + 1], scalar2=None,
                                            op0=Alu.add)

        def emit_fd(n):
            cols = slice(512 * n, 512 * (n + 1))
            pfq = _TL(psB, [16, 512], f32, tag="pB", padded_shape=[128, 512])
            for k in range(4):
                nc.tensor.matmul(pfq, wfdT[k][:, 0:16], xb[k][:, cols],
                                 start=(k == 0), stop=(k == 3))
            nc.scalar.activation(out=fqh16[:, cols], in_=pfq,
                                 func=Act.Identity, bias=fb16)
            pqd = _TL(psB, [8, 512], f32, tag="pB", padded_shape=[128, 512])
            for k in range(4):
                nc.tensor.matmul(pqd, wfdT[k][:, 16:24], xb[k][:, cols],
                                 start=(k == 0), stop=(k == 3))
            nc.scalar.activation(out=sig_sb[:, cols], in_=pqd,
                                 func=Act.Tanh, bias=bdh, scale=0.5)
            nc.vector.tensor_tensor(out=Qaux16[:, cols], in0=cs16[:, cols],
                                    in1=fqh16[:, cols], op=Alu.mult)

        def emit_ct(tt, j):
            rows = slice(128 * tt, 128 * (tt + 1))
            pc = _TL(psP, [128, 256], f32, tag="pP", padded_shape=[128, 512])
            for k in range(4):
                nc.tensor.matmul(pc, xb[k][:, rows], wcT[k],
                                 start=(k == 0), stop=(k == 3))
            if j % 2 == 0:
                nc.vector.tensor_copy(out=CT[:, tt, CT_C0:CT_C0 + 128], in_=pc[:, 0:128])
                nc.vector.tensor_copy(out=CT[:, tt, CT_C1:CT_C1 + 128], in_=pc[:, 128:256])
            else:
                nc.scalar.activation(out=CT[:, tt, CT_C0:CT_C0 + 128],
                                     in_=pc[:, 0:128], func=Act.Copy)
                nc.scalar.activation(out=CT[:, tt, CT_C1:CT_C1 + 128],
                                     in_=pc[:, 128:256], func=Act.Copy)

        def emit_gneg(i):
            pg = _TL(psB, [128, 2], f32, tag="pB", padded_shape=[128, 512])
            nc.tensor.matmul(pg, sig_sb[:, 128 * i:128 * (i + 1)], gco,
                             start=True, stop=True)
            nc.vector.tensor_scalar(out=gneg_sb[:, 2 * i:2 * i + 2], in0=pg,
                                    scalar1=-1.25, scalar2=None, op0=Alu.add)

        # ---- phase B: attention S + exp for one block ----
        def emit_diag(i):
            dg = _TL(work, [128, 2, 128], dt.float32r, tag="diag")
            for h in range(2):
                nc.gpsimd.tensor_scalar(out=dg[:, h, :], in0=I_sb,
                                        scalar1=gneg_sb[:, 2 * i + h:2 * i + h + 1],
                                        scalar2=None, op0=Alu.mult)
            return dg

        def emit_S(i, dg):
            s0, w0 = 128 * i, _w0_of_block(i)
            off = _off_of_block(i)
            s2t = _TL(psS, [128, 2, 512], f32, tag="pS", padded_shape=[128, 2, 512])
            for h in range(2):
                nc.tensor.matmul(s2t[:, h, 0:WIN], Q_sb[h][:, s0:s0 + 128],
                                 K_sb[h][:, w0:w0 + WIN], start=True, stop=False)
                nc.tensor.matmul(s2t[:, h, 0:WIN], Qaux16[:, s0:s0 + 128],
                                 csz[h][:, w0:w0 + WIN], start=False, stop=False)
                nc.tensor.matmul(s2t[:, h, 0:WIN], dg[:, h, :],
                                 d5[_pat_of_block(i)], start=False, stop=True)
            nc.scalar.activation(out=wn2[:, i, :, off:off + WIN],
                                 in_=s2t[:, :, 0:WIN], func=Act.Exp)

        # ---- phase B: PV (transposed) for one block ----
        def emit_pv(i):
            ks = _ks_of_block(i)
            pomh = []
            for h in range(2):
                pom = _TL(psB, [128, 137], f32, tag="pB", padded_shape=[128, 512])
                pomh.append(pom)
                rhs0 = CT_A0 if h == 0 else CT_A1
                for j, k in enumerate(ks):
                    nc.tensor.matmul(
                        pom, WnTu[:, i, h, k, :],
                        CT[:, i - 1 + k, rhs0:rhs0 + 137],
                        start=(j == 0), stop=(j == len(ks) - 1))
            for h in range(2):
                nc.vector.reciprocal(out=recip_sb[:, 2 * i + h:2 * i + h + 1],
                                     in_=pomh[h][:, 8:9])
            nc.vector.tensor_scalar(out=OMPT[:, i, 0, :], in0=pomh[0][:, 9:137],
                                     scalar1=recip_sb[:, 2 * i:2 * i + 1],
                                     scalar2=None, op0=Alu.mult)
            nc.scalar.activation(out=OMPT[:, i, 1, :], in_=pomh[1][:, 9:137],
                                 func=Act.Identity,
                                 scale=recip_sb[:, 2 * i + 1:2 * i + 2])
            for h in range(2):
                nc.vector.scalar_tensor_tensor(
                    out=OMPT[:, i, 2, 8 * h:8 * h + 8], in0=pomh[h][:, 0:8],
                    scalar=recip_sb[:, 2 * i + h:2 * i + h + 1],
                    in1=CT[:, i, CT_A0:CT_A0 + 8], op0=Alu.mult, op1=Alu.mult)

        def emit_wn_transpose1(i):
            nc.sync.dma_start_transpose(
                out=WnTu[:, i, :, :, :].rearrange("p hh k e -> p (hh k) e"),
                in_=wn2[:, i, :, :])

        def emit_post_transposes(p):
            nc.sync.dma_start_transpose(
                out=RES3[:, 2 * p:2 * p + 2, :, :],
                in_=OMPT[:, 2 * p:2 * p + 2, :, :])

        # ---- phase C: output projection for block pair p ----
        def emit_out(p):
            cols = slice(256 * p, 256 * (p + 1))
            for ot in range(4):
                osl = slice(128 * ot, 128 * (ot + 1))
                pp = _TL(psP, [128, 256], f32, tag="pP", padded_shape=[128, 512])
                nc.tensor.matmul(pp, wp12[0][:, osl], RES3[:, 2 * p:2 * p + 2, 0, :],
                                 start=True, stop=False)
                nc.tensor.matmul(pp, wp12[1][:, osl], RES3[:, 2 * p:2 * p + 2, 1, :],
                                 start=False, stop=False)
                nc.tensor.matmul(pp, wp3cat[:, osl],
                                 RES3[0:17, 2 * p:2 * p + 2, 2, :],
                                 start=False, stop=True)
                if ot % 2 == 0:
                    nc.vector.tensor_copy(out=OB[:, ot, cols], in_=pp)
                else:
                    nc.scalar.activation(out=OB[:, ot, cols], in_=pp, func=Act.Copy)

        def emit_store(g):
            cs_ = slice(512 * g, 512 * (g + 1))
            nc.sync.dma_start(
                out=out_d.ap().rearrange("(ot p) t -> p ot t", p=128)[:, :, cs_],
                in_=OB[:, :, cs_])

        def emit_store_pair(p):
            cs_ = slice(256 * p, 256 * (p + 1))
            nc.sync.dma_start(
                out=out_d.ap().rearrange("(ot p) t -> p ot t", p=128)[:, :, cs_],
                in_=OB[:, :, cs_])

        NP = NBLK // 2
        # prologue
        emit_qk(0)
        emit_fd(0)
        emit_ct(0, 0)
        emit_ct(1, 1)
        for i in range(4):
            emit_gneg(i)
        emit_qk(1)
        emit_fd(1)
        emit_ct(2, 2)
        emit_ct(3, 3)
        for i in range(4, 8):
            emit_gneg(i)
        dgq = [emit_diag(0), emit_diag(1)]
        for q in range(3):
            dgq.append(emit_diag(2 * q + 2))
            dgq.append(emit_diag(2 * q + 3))
            emit_S(2 * q, dgq[2 * q])
            emit_wn_transpose1(2 * q)
            emit_S(2 * q + 1, dgq[2 * q + 1])
            emit_wn_transpose1(2 * q + 1)
        for p in range(NP):
            if p == 0:
                emit_qk(2)
                emit_fd(2)
                for i in range(8, 12):
                    emit_gneg(i)
            if p == 2:
                emit_qk(3)
                emit_fd(3)
                for i in range(12, 16):
                    emit_gneg(i)
            if p < 6:
                emit_ct(2 * p + 4, 2 * p)
                emit_ct(2 * p + 5, 2 * p + 1)
            if p + 3 < NP:
                if 2 * p + 8 < NBLK:
                    dgq.append(emit_diag(2 * p + 8))
                if 2 * p + 9 < NBLK:
                    dgq.append(emit_diag(2 * p + 9))
                emit_S(2 * p + 6, dgq[2 * p + 6])
                emit_wn_transpose1(2 * p + 6)
                emit_S(2 * p + 7, dgq[2 * p + 7])
                emit_wn_transpose1(2 * p + 7)
            if p > 0:
                emit_out(p - 1)
            emit_pv(2 * p)
            emit_pv(2 * p + 1)
            emit_post_transposes(p)
            if p >= 3 and p % 2 == 1:
                emit_store((p - 3) // 2)
        emit_out(NP - 1)
        emit_store_pair(6)
        emit_store_pair(7)

        for pool in (work, psP, psB, psS, sing):
            pool.release()

    nc.compile()
    return nc


def _cos_sin():
    t = np.arange(T, dtype=np.float64)
    per = np.arange(1, NF + 1, dtype=np.float64)
    ang = 2 * math.pi * t[None, :] / per[:, None]
    return np.cos(ang).astype(np.float32), np.sin(ang).astype(np.float32)


def _d_patterns():
    d5 = np.empty((128, 3, WIN), np.float32)
    p = np.arange(128)[:, None]
    j = np.arange(WIN)[None, :]
    for k, off in enumerate((0, 64, 128)):
        d = np.abs(p + off - j).astype(np.float32)
        d[p + off == j] = DIAG_BIG
        d5[:, k, :] = d
    return d5


_COS, _SIN = _cos_sin()
_D5 = _d_patterns()


def _prep_core_inputs(inputs, b, hg):
    f32 = np.float32
    x_b = np.ascontiguousarray(np.asarray(inputs["x"])[b], dtype=f32)
    hsl = slice(hg * 256, (hg + 1) * 256)
    fsl = slice(hg * 8, (hg + 1) * 8)
    s = f32(1.0 / math.sqrt(128.0))
    cosT, sinT = _COS, _SIN

    Wq = np.asarray(inputs["W_query"], f32)[hsl] * s
    bq = np.asarray(inputs["b_query"], f32)[hsl] * s
    Wk = np.asarray(inputs["W_key"], f32)[hsl]
    bk = np.asarray(inputs["b_key"], f32)[hsl]
    Wc = np.asarray(inputs["W_content"], f32)[hsl]
    Wf = np.asarray(inputs["W_qfreq"], f32)[fsl] * f32(0.5)
    bf = np.asarray(inputs["b_qfreq"], f32)[fsl] * f32(0.5)
    Wd = np.asarray(inputs["W_qdecay"], f32)[fsl]
    bd = np.asarray(inputs["b_qdecay"], f32)[fsl]
    Wp = np.asarray(inputs["W_proj"], f32)
    Wp_hg = Wp[:, hg * 264:(hg + 1) * 264]

    # wfdT cols: [fq_h0 x2 (dup for cos/sin), fq_h1 x2, qd_h0, qd_h1]
    wfd = np.concatenate([Wf[0:4], Wf[0:4], Wf[4:8], Wf[4:8],
                          Wd[0:4], Wd[4:8]], axis=0)  # [24, 512]

    gco = np.zeros((8, 2), f32)
    dvec = -(np.arange(1, ND + 1, dtype=f32) / 8)
    gco[0:4, 0] = dvec
    gco[4:8, 1] = dvec

    wqkc = np.concatenate([Wq.T, Wk.T, Wc.T, wfd.T], axis=1)  # [512, 792]
    wp12 = np.stack([Wp_hg[:, 0:128].T, Wp_hg[:, 132:260].T])  # [2, 128, 512]

    # wp3cat rows: tsig proj (cos dup 0-3 + sin dup 4-7 per head) + bias row
    wp3cat = np.zeros((17, C), f32)
    wp3cat[0:8] = np.concatenate([Wp_hg[:, 128:132].T, Wp_hg[:, 128:132].T])
    wp3cat[8:16] = np.concatenate([Wp_hg[:, 260:264].T, Wp_hg[:, 260:264].T])
    if hg == 0:
        b_eff = np.asarray(inputs["b_proj"], f32).copy()
        bc = np.asarray(inputs["b_content"], f32)
        for h in range(HEADS):
            b_eff += Wp[:, 132 * h:132 * h + 128] @ bc[128 * h:128 * h + 128]
        wp3cat[16] = b_eff

    cs = np.concatenate([cosT, sinT], 0)  # [8, T]
    cs16 = np.concatenate([cs, cs], 0)    # [16, T]
    csT = np.ascontiguousarray(cs.T)      # [T, 8]
    # csT dup'd per head-aux: [128, tt, 2, 8]
    csTr = csT.reshape(16, 128, 8).transpose(1, 0, 2)  # [128, tt, 8]
    csT2 = np.concatenate([csTr[:, :, None, :], csTr[:, :, None, :]], axis=2)

    bigb = np.empty((128, XWCOLS), np.float32)
    bigb[:, XB0:WQ0] = x_b.reshape(4, 128, 2048).transpose(1, 0, 2).reshape(128, -1)
    bigb[:, WQ0:WP0] = wqkc.reshape(4, 128, 792).transpose(1, 0, 2).reshape(128, -1)
    bigb[:, WP0:CST0] = wp12.transpose(1, 0, 2).reshape(128, -1)
    bigb[:, CST0:XWCOLS] = csT2.reshape(128, -1)

    smallb = np.zeros((17, 6656), np.float32)
    smallb[0:16, 0:2048] = cs16
    smallb[0:8, 2048:4096] = cs
    smallb[8:16, 4096:6144] = cs
    smallb[:, 6144:6656] = wp3cat

    smallr = np.zeros((128, SRCOLS), f32)
    smallr[:, D50:D50 + 768] = _D5.reshape(128, -1)
    smallr[:, SR_I:SR_I + 128] = np.eye(128, dtype=f32)
    smallf = np.zeros((128, SFCOLS), f32)
    smallf[:, SF_BQ:SF_BQ + 2] = bq.reshape(2, 128).T
    smallf[:, SF_BK:SF_BK + 2] = bk.reshape(2, 128).T
    smallf[0:16, SF_FB] = np.concatenate([bf[0:4], bf[0:4], bf[4:8], bf[4:8]])
    smallf[0:8, SF_BD] = 0.5 * np.concatenate([bd[0:4], bd[4:8]])
    smallf[0:8, SF_GCO:SF_GCO + 2] = gco
    return {
        "bigb": bigb.astype(BF16),
        "smallb": smallb.astype(BF16),
        "smallf": smallf,
        "smallr": smallr,
    }


def get_nc():
    if "nc" not in _CACHE:
        _CACHE["nc"] = _build_nc()
    return _CACHE["nc"]


def make_in_maps(inputs):
    return [_prep_core_inputs(inputs, c // 2, c % 2) for c in range(8)]


def kernel(**inputs):
    from concourse.bass_utils import run_bass_kernel_spmd

    nc = get_nc()
    in_maps = make_in_maps(inputs)
    res = run_bass_kernel_spmd(nc, in_maps, core_ids=list(range(8)))
    x = np.asarray(inputs["x"], np.float32)
    out = np.empty((B, C, T), np.float32)
    for b in range(B):
        out[b] = res.results[2 * b]["out"].astype(np.float32)
        out[b] += res.results[2 * b + 1]["out"].astype(np.float32)
        out[b] += x[b]
    return out
